# revision 1
# baseline (speedup 1.0000x reference)
"""DiscreteMMSE Trainium2 Bass kernel.

Math (per batch row b):
  Z = data[b] @ W                      [N, T]   (W = squeeze(task_pool).T)
  resid = Z - targets[b][:, None]      [N, T]
  S'[i] = sum_{n<i} resid[n]^2         (strict cumsum over N; S'[0] = 0)
  E = exp(-0.5*S' - max_t(-0.5*S'))    (exact softmax-stable weights)
  out[b, i] = targets[b, i] + (sum_t E[i]*resid[i]) / (sum_t E[i])

Identical to the reference softmax-posterior MMSE prediction: the Gaussian
log-pdf constant and common shifts cancel in the softmax, and
pred = sum_t post*Z[i] = targets[i] - sum_t post*(targets[i]-Z[i]) collapses
onto resid. Row 0 (uniform prior over tasks) falls out of the strict cumsum.

Layout per NeuronCore (pure data parallel over B: 8 rows each, no collectives):
  - N=256 rows on partitions as two 128-row chunks; T=4096 on the free dim.
  - float32r (TF32-like, fp32 with 12 low mantissa bits dropped) matmuls are
    measured EXACT on f32r inputs and run 4x faster than fp32 matmuls, so
    every fp32 operand is split hi+lo into two f32r planes, making every
    matmul here fp32-exact at f32r speed:
      resid: lhsT = [data.T; targets] and rhs = [W; -1] each split hi/lo,
             3-term product (the lo*lo term is below fp32 resolution).
      cumsum input sq = resid^2: hi = f32r(sq) (GpSimd cast of the ScalarE
             Square output), lo = f32r(sq - hi) (GpSimd/VectorE split).
  - strict cumsum over N via triangular-ones f32r matmuls on TensorE:
    chunk0: U.T@{hi0,lo0} ; chunk1: U.T@{hi1,lo1} + ones.T@{hi0,lo0},
    accumulated in one PSUM group.
  - PSUM evacuation fused with the row-max: tensor_scalar(mult -0.5,
    accum max) on VectorE.
  - Exp on ScalarE with per-partition bias = -rowmax, accum_out = denominator.
  - numerator: resid recomputed (hi-term only, benign) into PSUM, E*=resid
    in place on VectorE, row-sum via in-place ScalarE Copy accum.
  - modulo-scheduled emission: engines execute their instruction streams
    IN ORDER, so per-jt rounds interleave batch b's stage-1 chain
    (resid->sq->hi/lo->cumsum->evac) with batch b-1's stage-2 chain
    (exp->recompute->mul->numsum); this keeps ready work at the front of
    every engine queue and hides all cross-engine round-trips.
"""

import numpy as np

B, N, D, T = 64, 256, 64, 4096
NCORES = 8
BPC = B // NCORES  # batch rows per core
NCH = 2            # partition chunks of N
PB = 128           # partitions per chunk
PT = 1024          # psum tile free size (2 banks)
MT = 512           # matmul moving free size (1 bank)
NJT = T // PT      # psum tiles per chunk row
NMM = PT // MT     # matmuls per psum tile

_cached_nc = None


def _build():
    import concourse.bacc as bacc
    import concourse.mybir as mybir
    import concourse.tile as tile
    from concourse import masks

    F32 = mybir.dt.float32
    F32R = mybir.dt.float32r
    AF = mybir.ActivationFunctionType
    OP = mybir.AluOpType

    nc = bacc.Bacc("TRN2", debug=False)
    data_d = nc.dram_tensor("data", (BPC, N, D), F32, kind="ExternalInput")
    targ_d = nc.dram_tensor("targets", (BPC, N), F32, kind="ExternalInput")
    pool_d = nc.dram_tensor("task_pool", (T, D), F32, kind="ExternalInput")
    out_d = nc.dram_tensor("out", (BPC, N), F32, kind="ExternalOutput")

    with tile.TileContext(nc) as tc:
        with tc.tile_pool(name="const", bufs=1) as const:
            utri = const.tile([PB, PB], F32R)     # strictly-upper ones (lhsT)
            onesm = const.tile([PB, PB], F32R)    # all-ones
            waug_h = const.tile([D + 1, T], F32R)       # f32r hi of [W ; -1]
            waug_l = const.tile([D + 1, T], F32R)       # f32r lo
            daug_h = const.tile([D + 1, BPC * N], F32R)  # hi of [data.T ; tgt]
            daug_l = const.tile([D + 1, BPC * N], F32R)  # lo
            tpart = [const.tile([PB, BPC], F32, name=f"tpart{c}", tag=f"tpart{c}") for c in range(NCH)]
            den = [const.tile([PB, BPC], F32, name=f"den{c}", tag=f"den{c}") for c in range(NCH)]
            num = [const.tile([PB, BPC], F32, name=f"num{c}", tag=f"num{c}") for c in range(NCH)]

            nc.any.memset(onesm[:].bitcast(F32), 1.0)

            # ---- setup: transpose task_pool and data into lhsT layouts ----
            with (
                tc.tile_pool(name="ld", bufs=1) as ld,
                tc.tile_pool(name="tps", bufs=4, space="PSUM") as tps,
            ):
                ident = ld.tile([PB, PB], F32, tag="ident", name="ident")
                masks.make_identity(nc, ident[:])
                utri_f = ld.tile([PB, PB], F32, tag="utri_f", name="utri_f")
                masks.make_upper_triangular(nc, utri_f[:], 1.0, diag=False)
                nc.vector.tensor_copy(utri[:], utri_f[:])
                waug = ld.tile([D + 1, T], F32, tag="waug", name="waug")
                daug = ld.tile([D + 1, BPC * N], F32, tag="daug", name="daug")
                nc.any.memset(waug[D : D + 1, :], -1.0)
                wbig = ld.tile([PB, (T // PB) * D], F32, tag="wbig", name="wbig")
                nc.sync.dma_start(
                    wbig[:].rearrange("p (k d) -> p k d", d=D),
                    pool_d[:].rearrange("(k p) d -> p k d", p=PB),
                )
                for k in range(T // PB):
                    pt = tps.tile([D, PB], F32, tag="pt", name="pt")
                    nc.tensor.transpose(pt[:], wbig[:, k * D : (k + 1) * D], ident[:])
                    nc.vector.tensor_copy(waug[0:D, k * PB : (k + 1) * PB], pt[:])
                for b in range(BPC):
                    nc.sync.dma_start(
                        daug[D : D + 1, b * N : (b + 1) * N], targ_d[b : b + 1, :]
                    )
                    dbig = ld.tile([PB, NCH * D], F32, tag=f"dbig{b % 2}", name="dbig")
                    nc.sync.dma_start(
                        dbig[:].rearrange("p (c d) -> p c d", d=D),
                        data_d[b].rearrange("(c p) d -> p c d", p=PB),
                    )
                    for c in range(NCH):
                        pt = tps.tile([D, PB], F32, tag="pt", name="pt")
                        nc.tensor.transpose(
                            pt[:], dbig[:, c * D : (c + 1) * D], ident[:]
                        )
                        nc.vector.tensor_copy(
                            daug[0:D, b * N + c * PB : b * N + (c + 1) * PB], pt[:]
                        )
                        tv = targ_d[b, c * PB : (c + 1) * PB].rearrange(
                            "(p one) -> p one", one=1
                        )
                        nc.sync.dma_start(tpart[c][:, b : b + 1], tv)
                nc.vector.tensor_copy(waug_h[:], waug[:])
                nc.vector.tensor_sub(waug_l[:], waug[:], waug_h[:].bitcast(F32))
                nc.vector.tensor_copy(daug_h[:], daug[:])
                nc.vector.tensor_sub(daug_l[:], daug[:], daug_h[:].bitcast(F32))

            # ---- main pipeline ----
            with (
                tc.tile_pool(name="sq32p", bufs=3) as sq32p,
                tc.tile_pool(name="hilo", bufs=2) as hilo,
                tc.tile_pool(name="avp", bufs=2) as avp,
                tc.tile_pool(name="ep", bufs=3) as ep,
                tc.tile_pool(name="small", bufs=4) as small,
                tc.tile_pool(name="rpp", bufs=2, space="PSUM") as rpp,
                tc.tile_pool(name="spp", bufs=2, space="PSUM") as spp,
            ):

                def s1_alloc(b):
                    av = [
                        avp.tile([PB, T], F32, tag=f"av{c}", name=f"av{c}")
                        for c in range(NCH)
                    ]
                    mx2 = [
                        small.tile([PB, NJT], F32, tag=f"mx2{c}", name=f"mx2{c}")
                        for c in range(NCH)
                    ]
                    return av, mx2

                def s1_round(b, jt, av, mx2):
                    """per-jt chain: resid -> sq -> hi/lo -> cumsum -> evac."""
                    js = slice(jt * PT, (jt + 1) * PT)
                    his, los = [], []
                    for c in range(NCH):
                        cs = slice(b * N + c * PB, b * N + (c + 1) * PB)
                        rp = rpp.tile([PB, PT], F32, tag="rp", name="rp")
                        for h in range(NMM):
                            lo_ = jt * PT + h * MT
                            wsl = slice(lo_, lo_ + MT)
                            osl = rp[:, h * MT : (h + 1) * MT]
                            nc.tensor.matmul(
                                osl, daug_h[:, cs], waug_h[:, wsl],
                                start=True, stop=False,
                            )
                            nc.tensor.matmul(
                                osl, daug_h[:, cs], waug_l[:, wsl],
                                start=False, stop=False,
                            )
                            nc.tensor.matmul(
                                osl, daug_l[:, cs], waug_h[:, wsl],
                                start=False, stop=True,
                            )
                        sq32 = sq32p.tile([PB, PT], F32, tag="sq32", name="sq32")
                        nc.scalar.activation(sq32[:], rp[:], AF.Square)
                        hi_t = hilo.tile([PB, PT], F32R, tag=f"hi{c}", name=f"hi{c}")
                        nc.gpsimd.tensor_copy(hi_t[:], sq32[:])
                        lo_t = hilo.tile([PB, PT], F32R, tag=f"lo{c}", name=f"lo{c}")
                        if c == 0 or jt <= 1:
                            nc.gpsimd.tensor_sub(
                                lo_t[:], sq32[:], hi_t[:].bitcast(F32)
                            )
                        else:
                            nc.vector.tensor_sub(
                                lo_t[:], sq32[:], hi_t[:].bitcast(F32)
                            )
                        his.append(hi_t)
                        los.append(lo_t)
                    for c in range(NCH):
                        sp = spp.tile([PB, PT], F32, tag="sp", name="sp")
                        for h in range(NMM):
                            hsl = slice(h * MT, (h + 1) * MT)
                            ssl = sp[:, hsl]
                            nc.tensor.matmul(
                                ssl, utri[:], his[c][:, hsl],
                                start=True, stop=False,
                            )
                            nc.tensor.matmul(
                                ssl, utri[:], los[c][:, hsl],
                                start=False, stop=(c == 0),
                            )
                            if c == 1:
                                nc.tensor.matmul(
                                    ssl, onesm[:], his[0][:, hsl],
                                    start=False, stop=False,
                                )
                                nc.tensor.matmul(
                                    ssl, onesm[:], los[0][:, hsl],
                                    start=False, stop=True,
                                )
                        nc.vector.tensor_scalar(
                            out=av[c][:, js],
                            in0=sp[:],
                            scalar1=-0.5,
                            scalar2=None,
                            op0=OP.mult,
                            op1=OP.max,
                            accum_out=mx2[c][:, jt : jt + 1],
                        )

                def s1_finish(b, mx2):
                    """negated row-max once all evac partials of b landed."""
                    nbs = []
                    for c in range(NCH):
                        nb = small.tile([PB, 1], F32, tag=f"nb{c}", name=f"nb{c}")
                        nc.vector.tensor_reduce(
                            nb[:], mx2[c][:], axis=mybir.AxisListType.X, op=OP.max,
                            negate=True,
                        )
                        nbs.append(nb)
                    return nbs

                def s2_alloc(b):
                    den4 = [
                        small.tile([PB, NJT], F32, tag=f"den4{c}", name=f"den4{c}")
                        for c in range(NCH)
                    ]
                    num4 = [
                        small.tile([PB, NJT], F32, tag=f"num4{c}", name=f"num4{c}")
                        for c in range(NCH)
                    ]
                    return den4, num4

                def s2_round(b, jt, av, nbs, den4, num4):
                    """exp -> resid recompute -> E*resid -> numsum for (b, jt)."""
                    js = slice(jt * PT, (jt + 1) * PT)
                    for c in range(NCH):
                        ev = ep.tile([PB, PT], F32, tag=f"E{c}", name=f"E{c}")
                        nc.scalar.activation(
                            ev[:],
                            av[c][:, js],
                            AF.Exp,
                            bias=nbs[c][:],
                            scale=1.0,
                            accum_out=den4[c][:, jt : jt + 1],
                        )
                        lhsT_r = daug_h[:, b * N + c * PB : b * N + (c + 1) * PB]
                        rp2 = spp.tile([PB, PT], F32, tag="sp", name="rp2")
                        for h in range(NMM):
                            lo_ = jt * PT + h * MT
                            nc.tensor.matmul(
                                rp2[:, h * MT : (h + 1) * MT],
                                lhsT_r,
                                waug_h[:, lo_ : lo_ + MT],
                            )
                        nc.vector.tensor_mul(ev[:], ev[:], rp2[:])
                        nc.vector.tensor_scalar(
                            out=ev[:],
                            in0=ev[:],
                            scalar1=1.0,
                            scalar2=None,
                            op0=OP.mult,
                            op1=OP.add,
                            accum_out=num4[c][:, jt : jt + 1],
                        )

                def s2_finish(b, den4, num4):
                    for c in range(NCH):
                        nc.vector.tensor_reduce(
                            den[c][:, b : b + 1], den4[c][:],
                            axis=mybir.AxisListType.X, op=OP.add,
                        )
                        nc.vector.tensor_reduce(
                            num[c][:, b : b + 1], num4[c][:],
                            axis=mybir.AxisListType.X, op=OP.add,
                        )

                # modulo-scheduled pipeline: per-jt rounds interleave batch b's
                # stage-1 chain with batch b-1's stage-2 chain so each engine's
                # in-order stream always has ready work at the front.
                prev = None
                for b in range(BPC):
                    av, mx2 = s1_alloc(b)
                    if prev is not None:
                        pb, pav, pnbs, pden4, pnum4 = prev
                    for jt in range(NJT):
                        if prev is not None:
                            s2_round(pb, jt, pav, pnbs, pden4, pnum4)
                        s1_round(b, jt, av, mx2)
                    if prev is not None:
                        s2_finish(pb, pden4, pnum4)
                    nbs = s1_finish(b, mx2)
                    den4, num4 = s2_alloc(b)
                    prev = (b, av, nbs, den4, num4)
                pb, pav, pnbs, pden4, pnum4 = prev
                for jt in range(NJT):
                    s2_round(pb, jt, pav, pnbs, pden4, pnum4)
                s2_finish(pb, pden4, pnum4)

                # finals: out = targets + num/den
                for c in range(NCH):
                    rec = small.tile([PB, BPC], F32, tag=f"rec{c}", name=f"rec{c}")
                    prod = small.tile([PB, BPC], F32, tag=f"prod{c}", name=f"prod{c}")
                    outv = small.tile([PB, BPC], F32, tag=f"outv{c}", name=f"outv{c}")
                    nc.vector.reciprocal(rec[:], den[c][:])
                    nc.vector.tensor_mul(prod[:], num[c][:], rec[:])
                    nc.vector.tensor_add(outv[:], tpart[c][:], prod[:])
                    ov = out_d[:, c * PB : (c + 1) * PB].rearrange("b p -> p b")
                    nc.sync.dma_start(ov, outv[:])

    nc.compile()
    return nc


def _get_nc():
    global _cached_nc
    if _cached_nc is None:
        _cached_nc = _build()
    return _cached_nc


_cached_runner = None


def _get_runner():
    """Build once: a cached jax.jit shard_map over the 8 NeuronCores.

    run_bass_kernel_spmd/run_bass_via_pjrt construct a fresh jax.jit closure
    per call (full retrace); caching the callable keeps repeat calls cheap.
    """
    global _cached_runner
    if _cached_runner is None:
        import jax
        from jax.sharding import Mesh, PartitionSpec
        from concourse import bass2jax
        from concourse.bass2jax import _bass_exec_p, partition_id_tensor
        import concourse.mybir as mybir

        try:
            from jax.experimental.shard_map import shard_map
        except ImportError:
            from jax.shard_map import shard_map

        bass2jax.install_neuronx_cc_hook()
        nc = _get_nc()
        partition_name = (
            nc.partition_id_tensor.name if nc.partition_id_tensor else None
        )
        in_names, out_names, out_avals, zero_outs = [], [], [], []
        for alloc in nc.m.functions[0].allocations:
            if not isinstance(alloc, mybir.MemoryLocationSet):
                continue
            name = alloc.memorylocations[0].name
            if alloc.kind == "ExternalInput":
                if name != partition_name:
                    in_names.append(name)
            elif alloc.kind == "ExternalOutput":
                out_names.append(name)
                shape = tuple(alloc.tensor_shape)
                dtype = mybir.dt.np(alloc.dtype)
                out_avals.append(jax.core.ShapedArray(shape, dtype))
                zero_outs.append(np.zeros((NCORES * shape[0], *shape[1:]), dtype))
        n_params = len(in_names)
        all_names = list(in_names) + list(out_names)
        if partition_name is not None:
            all_names.append(partition_name)
        donate = tuple(range(n_params, n_params + len(out_names)))

        def _body(*args):
            operands = list(args)
            if partition_name is not None:
                operands.append(partition_id_tensor())
            return tuple(
                _bass_exec_p.bind(
                    *operands,
                    out_avals=tuple(out_avals),
                    in_names=tuple(all_names),
                    out_names=tuple(out_names),
                    lowering_input_output_aliases=(),
                    sim_require_finite=True,
                    sim_require_nnan=True,
                    nc=nc,
                )
            )

        devices = jax.devices()[:NCORES]
        mesh = Mesh(np.asarray(devices), ("core",))
        in_specs = tuple(
            PartitionSpec() if name == "task_pool" else PartitionSpec("core")
            for name in in_names
        ) + (PartitionSpec("core"),) * len(out_names)
        sharded = jax.jit(
            shard_map(
                _body,
                mesh=mesh,
                in_specs=in_specs,
                out_specs=(PartitionSpec("core"),) * len(out_names),
                check_rep=False,
            ),
            donate_argnums=donate,
            keep_unused=True,
        )
        _cached_runner = (sharded, in_names, out_names, out_avals, zero_outs)
    return _cached_runner


def _kernel_fallback(data, targets, tp):
    """Robust path via the stock SPMD runner (fresh jit each call)."""
    from concourse.bass_utils import run_bass_kernel_spmd

    nc = _get_nc()
    in_maps = [
        {
            "data": data[i * BPC : (i + 1) * BPC],
            "targets": targets[i * BPC : (i + 1) * BPC],
            "task_pool": tp,
        }
        for i in range(NCORES)
    ]
    res = run_bass_kernel_spmd(nc, in_maps, core_ids=list(range(NCORES)))
    return np.concatenate([r["out"] for r in res.results], axis=0)


def kernel(data, targets, task_pool, **_):
    data = np.ascontiguousarray(np.asarray(data, np.float32))
    targets = np.ascontiguousarray(np.asarray(targets, np.float32))
    tp = np.ascontiguousarray(np.asarray(task_pool, np.float32).reshape(T, D))

    try:
        sharded, in_names, out_names, out_avals, zero_outs = _get_runner()
        full = {
            "data": data.reshape(NCORES * BPC, N, D),
            "targets": targets.reshape(NCORES * BPC, N),
            "task_pool": tp,
        }
        args = [full[name] for name in in_names]
        args += [np.zeros_like(z) for z in zero_outs]
        outs = sharded(*args)
        out = np.asarray(outs[out_names.index("out")])
        return out.reshape(B, N)
    except Exception:
        return _kernel_fallback(data, targets, tp)



# revision 12
# speedup vs baseline: 1.5145x; 1.5145x over previous
"""DiscreteMMSE Trainium2 Bass kernel.

Math (per batch row b):
  Z = data[b] @ W                      [N, T]   (W = squeeze(task_pool).T)
  resid = Z - targets[b][:, None]      [N, T]
  S'[i] = sum_{n<i} resid[n]^2         (strict cumsum over N; S'[0] = 0)
  E = exp(-0.5*S' - max_t(-0.5*S'))    (softmax-stable weights)
  out[b, i] = (sum_t E[i]*Z[i]) / (sum_t E[i])

Identical to the reference softmax-posterior MMSE prediction: the Gaussian
log-pdf constant and common shifts cancel in the softmax; pred is the
posterior-weighted mean of the per-task predictions Z. Row 0 (uniform prior
over tasks) falls out of the strict cumsum (S'[0] = 0 => uniform weights).

Numerics: plain f32r (TF32-like) matmuls throughout. Measured end-to-end
rel_l2 ~ 9.5e-3 vs the fp32 reference (tolerance 2e-2): the f32r input
rounding perturbs logits by ~+-0.4 which the 4096-task posterior average
absorbs. This halves TensorE work and removes all hi/lo split traffic
(~200us of Pool/DVE busy) vs the exact-fp32 variant.

Hardware constraints (verified against the BIR verifier) that dictate the
engine split: GPSIMD/Pool cannot touch PSUM at all; DVE cannot read two
PSUM operands (so it cannot square a PSUM tile); DMA cannot address PSUM;
only Act can square straight out of PSUM; f32r matmul inputs must come
from rounding-capable producers (engine cast copies - never DMA/bitcast).
Six [128,1024] PSUM tiles (resid x2, cumsum x2, Z x2) must therefore be
egressed per round by Act+DVE alone - each egress is fused with its
compute so no pass is pure data movement.

Layout per NeuronCore (pure data parallel over B: 8 rows each, no
collectives): N=256 steps on partitions as two 128-row chunks (c=0,1);
T=4096 tasks on the free dim in four 1024-col tiles (jt). Per round
(both chunks of one jt), cost-model busy ns:
  PE   : resid matmuls (lhsT=[data.T;tgt], rhs=[W;-1], K=65, f32r),
         strict-cumsum via triangular matmul (+ ones-matmul chunk0
         column-sum offset into chunk1), Z recompute (K=64)    [2989]
  Act  : Square c0+c1 (PSUM resid -> SBUF sq f32r); one batched
         [128,4096] Exp per (chunk, b) (av -> E bf16, bias=-rowmax,
         accum_out writes den[c][:,b] directly)                [~4343]
  DVE  : both evacs (PSUM cum * -0.5, fused row-max accum -> av),
         both scalar_tensor_tensor (E*Z fused with row-sum accum ->
         num partials; one instruction replaces mul+sum)       [~4800]
Modulo-scheduled: per-jt rounds interleave batch b's stage-1 chain with
batch b-1's stage-2 chain so each engine's in-order queue stays fed.
"""

import numpy as np

B, N, D, T = 64, 256, 64, 4096
NCORES = 8
BPC = B // NCORES  # batch rows per core
NCH = 2            # partition chunks of N
PB = 128           # partitions per chunk
PT = 1024          # psum tile free size (2 banks)
MT = 512           # matmul moving free size (1 bank)
NJT = T // PT      # psum tiles per chunk row
NMM = PT // MT     # matmuls per psum tile

_cached_nc = None


def _build():
    import concourse.bacc as bacc
    import concourse.mybir as mybir
    import concourse.tile as tile
    from concourse import masks

    F32 = mybir.dt.float32
    F32R = mybir.dt.float32r
    BF16 = mybir.dt.bfloat16
    AF = mybir.ActivationFunctionType
    OP = mybir.AluOpType

    nc = bacc.Bacc("TRN2", debug=False)
    data_d = nc.dram_tensor("data", (BPC, N, D), F32, kind="ExternalInput")
    targ_d = nc.dram_tensor("targets", (BPC, N), F32, kind="ExternalInput")
    pool_d = nc.dram_tensor("task_pool", (T, D), F32, kind="ExternalInput")
    out_d = nc.dram_tensor("out", (BPC, N), F32, kind="ExternalOutput")

    with tile.TileContext(nc) as tc:
        with tc.tile_pool(name="const", bufs=1) as const:
            utri = const.tile([PB, PB], F32R)     # strictly-upper ones (lhsT)
            onesm = const.tile([PB, PB], F32R)    # all-ones
            waug = const.tile([D + 1, T], F32R)        # [W ; -1]
            daug = const.tile([D + 1, BPC * N], F32R)  # [data.T ; tgt]
            den = [const.tile([PB, BPC], F32, name=f"den{c}", tag=f"den{c}") for c in range(NCH)]
            num = [const.tile([PB, BPC], F32, name=f"num{c}", tag=f"num{c}") for c in range(NCH)]

            nc.any.memset(onesm[:].bitcast(F32), 1.0)
            nc.any.memset(waug[D : D + 1, :].bitcast(F32), -1.0)

            # ---- setup: transpose task_pool and data into lhsT layouts ----
            with (
                tc.tile_pool(name="ld", bufs=1) as ld,
                tc.tile_pool(name="tps", bufs=4, space="PSUM") as tps,
            ):
                ident = ld.tile([PB, PB], F32, tag="ident", name="ident")
                masks.make_identity(nc, ident[:])
                utri_f = ld.tile([PB, PB], F32, tag="utri_f", name="utri_f")
                masks.make_upper_triangular(nc, utri_f[:], 1.0, diag=False)
                nc.vector.tensor_copy(utri[:], utri_f[:])
                tstag = ld.tile([1, BPC * N], F32, tag="tstag", name="tstag")
                wbig = ld.tile([PB, (T // PB) * D], F32, tag="wbig", name="wbig")
                nc.sync.dma_start(
                    wbig[:].rearrange("p (k d) -> p k d", d=D),
                    pool_d[:].rearrange("(k p) d -> p k d", p=PB),
                )
                TB = 4  # transposes batched per PSUM tile
                # one contiguous targets DMA; daug row D gates the first resid
                nc.sync.dma_start(
                    tstag[:],
                    targ_d[:].rearrange("b n -> (b n)").rearrange(
                        "(one m) -> one m", one=1
                    ),
                )
                nc.scalar.activation(daug[D : D + 1, :], tstag[:], AF.Copy)
                dbigs = []
                for b in range(BPC):
                    dbig = ld.tile(
                        [PB, NCH * D], F32, tag=f"dbig{b}", name="dbig"
                    )
                    nc.sync.dma_start(
                        dbig[:].rearrange("p (c d) -> p c d", d=D),
                        data_d[b].rearrange("(c p) d -> p c d", p=PB),
                    )
                    dbigs.append(dbig)
                for g in range(BPC * NCH // TB):
                    pt = tps.tile([D, TB * PB], F32, tag="pt", name="pt")
                    for i in range(TB):
                        bc = g * TB + i
                        b, c = bc // NCH, bc % NCH
                        nc.tensor.transpose(
                            pt[:, i * PB : (i + 1) * PB],
                            dbigs[b][:, c * D : (c + 1) * D],
                            ident[:],
                        )
                    nc.scalar.activation(
                        daug[0:D, g * TB * PB : (g + 1) * TB * PB], pt[:], AF.Copy
                    )
                for g in range(T // PB // TB):
                    pt = tps.tile([D, TB * PB], F32, tag="pt", name="pt")
                    for i in range(TB):
                        k = g * TB + i
                        nc.tensor.transpose(
                            pt[:, i * PB : (i + 1) * PB],
                            wbig[:, k * D : (k + 1) * D],
                            ident[:],
                        )
                    nc.scalar.activation(
                        waug[0:D, g * TB * PB : (g + 1) * TB * PB], pt[:], AF.Copy
                    )

            # ---- main pipeline ----
            with (
                tc.tile_pool(name="sqp", bufs=4) as sqp,
                tc.tile_pool(name="avp", bufs=2) as avp,
                tc.tile_pool(name="ep", bufs=3) as ep,
                tc.tile_pool(name="small", bufs=4) as small,
                tc.tile_pool(name="rpp", bufs=2, space="PSUM") as rpp,
                tc.tile_pool(name="spp", bufs=2, space="PSUM") as spp,
            ):

                def s1_alloc(b):
                    av = [
                        avp.tile([PB, T], F32, tag=f"av{c}", name=f"av{c}")
                        for c in range(NCH)
                    ]
                    mx4 = [
                        small.tile([PB, NJT], F32, tag=f"mx4{c}", name=f"mx4{c}")
                        for c in range(NCH)
                    ]
                    return av, mx4

                def s1_resid(b, jt, c):
                    """resid matmul -> PSUM; Act square (fused PSUM egress)."""
                    cs = slice(b * N + c * PB, b * N + (c + 1) * PB)
                    rp = rpp.tile([PB, PT], F32, tag="rp", name="rp")
                    for h in range(NMM):
                        lo_ = jt * PT + h * MT
                        nc.tensor.matmul(
                            rp[:, h * MT : (h + 1) * MT],
                            daug[:, cs],
                            waug[:, lo_ : lo_ + MT],
                            start=True,
                            stop=True,
                        )
                    sq = sqp.tile([PB, PT], F32R, tag=f"sq{c}", name=f"sq{c}")
                    nc.scalar.activation(sq[:], rp[:], AF.Square)
                    return sq

                def s1_cum(b, jt, c, sqs, av, mx4):
                    """strict cumsum + evac(-0.5, fused row-max) on DVE."""
                    js = slice(jt * PT, (jt + 1) * PT)
                    sp = spp.tile([PB, PT], F32, tag="sp", name="sp")
                    for h in range(NMM):
                        hsl = slice(h * MT, (h + 1) * MT)
                        nc.tensor.matmul(
                            sp[:, hsl], utri[:], sqs[c][:, hsl],
                            start=True, stop=(c == 0),
                        )
                        if c == 1:
                            nc.tensor.matmul(
                                sp[:, hsl], onesm[:], sqs[0][:, hsl],
                                start=False, stop=True,
                            )
                    nc.vector.tensor_scalar(
                        out=av[c][:, js],
                        in0=sp[:],
                        scalar1=-0.5,
                        scalar2=None,
                        op0=OP.mult,
                        op1=OP.max,
                        accum_out=mx4[c][:, jt : jt + 1],
                    )

                def s1_finish(b, mx4):
                    """negated row-max once all evac partials of b landed."""
                    nbs = []
                    for c in range(NCH):
                        nb = small.tile([PB, 1], F32, tag=f"nb{c}", name=f"nb{c}")
                        nc.vector.tensor_reduce(
                            nb[:], mx4[c][:], axis=mybir.AxisListType.X, op=OP.max,
                            negate=True,
                        )
                        nbs.append(nb)
                    return nbs

                def s2_exp(b, jt, c, av, nbs, den4):
                    """E = exp(av - rowmax) -> bf16, den partial via accum."""
                    js = slice(jt * PT, (jt + 1) * PT)
                    ev = ep.tile([PB, PT], BF16, tag=f"E{c}", name=f"E{c}")
                    nc.scalar.activation(
                        ev[:],
                        av[c][:, js],
                        AF.Exp,
                        bias=nbs[c][:],
                        scale=1.0,
                        accum_out=den4[c][:, jt : jt + 1],
                    )
                    return ev

                def s2_num(b, jt, c, ev, num4):
                    """Z recompute (K=64) + fused E*Z row-sum on DVE."""
                    rp2 = spp.tile([PB, PT], F32, tag="sp", name="rp2")
                    lhsT_z = daug[0:D, b * N + c * PB : b * N + (c + 1) * PB]
                    for h in range(NMM):
                        lo_ = jt * PT + h * MT
                        nc.tensor.matmul(
                            rp2[:, h * MT : (h + 1) * MT],
                            lhsT_z,
                            waug[0:D, lo_ : lo_ + MT],
                        )
                    nc.vector.scalar_tensor_tensor(
                        out=ev[:],
                        in0=rp2[:],
                        scalar=1.0,
                        in1=ev[:],
                        op0=OP.mult,
                        op1=OP.mult,
                        accum_out=num4[c][:, jt : jt + 1],
                    )

                def s2_alloc(b):
                    den4 = [
                        small.tile([PB, NJT], F32, tag=f"den4{c}", name=f"den4{c}")
                        for c in range(NCH)
                    ]
                    num4 = [
                        small.tile([PB, NJT], F32, tag=f"num4{c}", name=f"num4{c}")
                        for c in range(NCH)
                    ]
                    return den4, num4

                def s2_finish(b, den4, num4):
                    for c in range(NCH):
                        nc.vector.tensor_reduce(
                            den[c][:, b : b + 1], den4[c][:],
                            axis=mybir.AxisListType.X, op=OP.add,
                        )
                        nc.vector.tensor_reduce(
                            num[c][:, b : b + 1], num4[c][:],
                            axis=mybir.AxisListType.X, op=OP.add,
                        )

                # modulo-scheduled pipeline: per-jt rounds interleave batch b's
                # stage-1 chain with batch b-1's stage-2 chain so each engine's
                # in-order stream always has ready work at the front.
                def round_(b, jt, av, mx4, prev):
                    if prev is not None:
                        pb, pav, pnbs, pden4, pnum4 = prev
                    if jt == 0:
                        # new-b squares first: the old-b exp waits on the
                        # row-max reduce and must not block them on Act
                        sq0 = s1_resid(b, jt, 0)
                        sq1 = s1_resid(b, jt, 1)
                        if prev is not None:
                            ev0 = s2_exp(pb, jt, 0, pav, pnbs, pden4)
                            ev1 = s2_exp(pb, jt, 1, pav, pnbs, pden4)
                    else:
                        if prev is not None:
                            ev0 = s2_exp(pb, jt, 0, pav, pnbs, pden4)
                        sq0 = s1_resid(b, jt, 0)
                        if prev is not None:
                            ev1 = s2_exp(pb, jt, 1, pav, pnbs, pden4)
                        sq1 = s1_resid(b, jt, 1)
                    sqs = [sq0, sq1]
                    if prev is not None:
                        s2_num(pb, jt, 0, ev0, pnum4)
                    s1_cum(b, jt, 0, sqs, av, mx4)
                    if prev is not None:
                        s2_num(pb, jt, 1, ev1, pnum4)
                    s1_cum(b, jt, 1, sqs, av, mx4)

                prev = None
                for b in range(BPC):
                    av, mx4 = s1_alloc(b)
                    for jt in range(NJT):
                        round_(b, jt, av, mx4, prev)
                    if prev is not None:
                        s2_finish(prev[0], prev[3], prev[4])
                    nbs = s1_finish(b, mx4)
                    den4, num4 = s2_alloc(b)
                    prev = (b, av, nbs, den4, num4)
                pb, pav, pnbs, pden4, pnum4 = prev
                for jt in range(NJT):
                    ev0 = s2_exp(pb, jt, 0, pav, pnbs, pden4)
                    s2_num(pb, jt, 0, ev0, pnum4)
                    ev1 = s2_exp(pb, jt, 1, pav, pnbs, pden4)
                    s2_num(pb, jt, 1, ev1, pnum4)
                s2_finish(pb, pden4, pnum4)

                # finals: out = num/den
                for c in range(NCH):
                    rec = small.tile([PB, BPC], F32, tag=f"rec{c}", name=f"rec{c}")
                    outv = small.tile([PB, BPC], F32, tag=f"outv{c}", name=f"outv{c}")
                    nc.vector.reciprocal(rec[:], den[c][:])
                    nc.vector.tensor_mul(outv[:], num[c][:], rec[:])
                    ov = out_d[:, c * PB : (c + 1) * PB].rearrange("b p -> p b")
                    nc.sync.dma_start(ov, outv[:])

    nc.compile()
    return nc


def _get_nc():
    global _cached_nc
    if _cached_nc is None:
        _cached_nc = _build()
    return _cached_nc


_cached_runner = None


def _get_runner():
    """Build once: a cached jax.jit shard_map over the 8 NeuronCores.

    run_bass_kernel_spmd/run_bass_via_pjrt construct a fresh jax.jit closure
    per call (full retrace); caching the callable keeps repeat calls cheap.
    """
    global _cached_runner
    if _cached_runner is None:
        import jax
        from jax.sharding import Mesh, PartitionSpec
        from concourse import bass2jax
        from concourse.bass2jax import _bass_exec_p, partition_id_tensor
        import concourse.mybir as mybir

        try:
            from jax.experimental.shard_map import shard_map
        except ImportError:
            from jax.shard_map import shard_map

        bass2jax.install_neuronx_cc_hook()
        nc = _get_nc()
        partition_name = (
            nc.partition_id_tensor.name if nc.partition_id_tensor else None
        )
        in_names, out_names, out_avals, zero_outs = [], [], [], []
        for alloc in nc.m.functions[0].allocations:
            if not isinstance(alloc, mybir.MemoryLocationSet):
                continue
            name = alloc.memorylocations[0].name
            if alloc.kind == "ExternalInput":
                if name != partition_name:
                    in_names.append(name)
            elif alloc.kind == "ExternalOutput":
                out_names.append(name)
                shape = tuple(alloc.tensor_shape)
                dtype = mybir.dt.np(alloc.dtype)
                out_avals.append(jax.core.ShapedArray(shape, dtype))
                zero_outs.append(np.zeros((NCORES * shape[0], *shape[1:]), dtype))
        n_params = len(in_names)
        all_names = list(in_names) + list(out_names)
        if partition_name is not None:
            all_names.append(partition_name)
        donate = tuple(range(n_params, n_params + len(out_names)))

        def _body(*args):
            operands = list(args)
            if partition_name is not None:
                operands.append(partition_id_tensor())
            return tuple(
                _bass_exec_p.bind(
                    *operands,
                    out_avals=tuple(out_avals),
                    in_names=tuple(all_names),
                    out_names=tuple(out_names),
                    lowering_input_output_aliases=(),
                    sim_require_finite=True,
                    sim_require_nnan=True,
                    nc=nc,
                )
            )

        devices = jax.devices()[:NCORES]
        mesh = Mesh(np.asarray(devices), ("core",))
        in_specs = tuple(
            PartitionSpec() if name == "task_pool" else PartitionSpec("core")
            for name in in_names
        ) + (PartitionSpec("core"),) * len(out_names)
        sharded = jax.jit(
            shard_map(
                _body,
                mesh=mesh,
                in_specs=in_specs,
                out_specs=(PartitionSpec("core"),) * len(out_names),
                check_rep=False,
            ),
            donate_argnums=donate,
            keep_unused=True,
        )
        _cached_runner = (sharded, in_names, out_names, out_avals, zero_outs)
    return _cached_runner


def _kernel_fallback(data, targets, tp):
    """Robust path via the stock SPMD runner (fresh jit each call)."""
    from concourse.bass_utils import run_bass_kernel_spmd

    nc = _get_nc()
    in_maps = [
        {
            "data": data[i * BPC : (i + 1) * BPC],
            "targets": targets[i * BPC : (i + 1) * BPC],
            "task_pool": tp,
        }
        for i in range(NCORES)
    ]
    res = run_bass_kernel_spmd(nc, in_maps, core_ids=list(range(NCORES)))
    return np.concatenate([r["out"] for r in res.results], axis=0)


def kernel(data, targets, task_pool, **_):
    data = np.ascontiguousarray(np.asarray(data, np.float32))
    targets = np.ascontiguousarray(np.asarray(targets, np.float32))
    tp = np.ascontiguousarray(np.asarray(task_pool, np.float32).reshape(T, D))

    try:
        sharded, in_names, out_names, out_avals, zero_outs = _get_runner()
        full = {
            "data": data.reshape(NCORES * BPC, N, D),
            "targets": targets.reshape(NCORES * BPC, N),
            "task_pool": tp,
        }
        args = [full[name] for name in in_names]
        args += [np.zeros_like(z) for z in zero_outs]
        outs = sharded(*args)
        out = np.asarray(outs[out_names.index("out")])
        return out.reshape(B, N)
    except Exception:
        return _kernel_fallback(data, targets, tp)


# revision 19
# speedup vs baseline: 1.5576x; 1.0285x over previous
"""DiscreteMMSE Trainium2 Bass kernel.

Math (per batch row b):
  Z = data[b] @ W                      [N, T]   (W = squeeze(task_pool).T)
  resid = Z - targets[b][:, None]      [N, T]
  S'[i] = sum_{n<i} resid[n]^2         (strict cumsum over N; S'[0] = 0)
  E = exp(-0.5*S' - max_t(-0.5*S'))    (softmax-stable weights)
  out[b, i] = (sum_t E[i]*Z[i]) / (sum_t E[i])

Identical to the reference softmax-posterior MMSE prediction: the Gaussian
log-pdf constant and common shifts cancel in the softmax; pred is the
posterior-weighted mean of the per-task predictions Z. Row 0 (uniform prior
over tasks) falls out of the strict cumsum (S'[0] = 0 => uniform weights).

Numerics: plain f32r (TF32-like) matmuls throughout. Measured end-to-end
rel_l2 = 5.7e-3 on device vs the fp32 reference (tolerance 2e-2): the f32r
input rounding perturbs logits by ~+-0.4 which the 4096-task posterior
average absorbs. This halves TensorE work and removes all hi/lo split traffic
(~200us of Pool/DVE busy) vs the exact-fp32 variant.

Hardware constraints (verified against the BIR verifier) that dictate the
engine split: GPSIMD/Pool cannot touch PSUM at all; DVE cannot read two
PSUM operands (so it cannot square a PSUM tile); DMA cannot address PSUM;
only Act can square straight out of PSUM; f32r matmul inputs must come
from rounding-capable producers (engine cast copies - never DMA/bitcast).
Six [128,1024] PSUM tiles (resid x2, cumsum x2, Z x2) must therefore be
egressed per round by Act+DVE alone - each egress is fused with its
compute so no pass is pure data movement.

Layout per NeuronCore (pure data parallel over B: 8 rows each, no
collectives): N=256 steps on partitions as two 128-row chunks (c=0,1);
T=4096 tasks on the free dim in four 1024-col tiles (jt). Per round
(both chunks of one jt), cost-model busy ns:
  PE   : resid matmuls (lhsT=[data.T;tgt], rhs=[W;-1], K=65, f32r),
         strict-cumsum via triangular matmul (+ ones-matmul chunk0
         column-sum offset into chunk1), Z recompute (K=64)    [2989]
  Act  : Square c0+c1 (PSUM resid -> SBUF sq f32r); one batched
         [128,4096] Exp per (chunk, b) (av -> E bf16, bias=-rowmax,
         accum_out writes den[c][:,b] directly)                [~4343]
  DVE  : both evacs (PSUM cum * -0.5, fused row-max accum -> av),
         both scalar_tensor_tensor (E*Z fused with row-sum accum ->
         num partials; one instruction replaces mul+sum)       [~4800]
Modulo-scheduled: per-jt rounds interleave batch b's stage-1 chain with
batch b-1's stage-2 chain so each engine's in-order queue stays fed.
"""

import numpy as np

B, N, D, T = 64, 256, 64, 4096
NCORES = 8
BPC = B // NCORES  # batch rows per core
NCH = 2            # partition chunks of N
PB = 128           # partitions per chunk
PT = 1024          # psum tile free size (2 banks)
MT = 512           # matmul moving free size (1 bank)
NJT = T // PT      # psum tiles per chunk row
NMM = PT // MT     # matmuls per psum tile

_cached_nc = None


def _build():
    import concourse.bacc as bacc
    import concourse.mybir as mybir
    import concourse.tile as tile
    from concourse import masks

    F32 = mybir.dt.float32
    F32R = mybir.dt.float32r
    BF16 = mybir.dt.bfloat16
    AF = mybir.ActivationFunctionType
    OP = mybir.AluOpType

    nc = bacc.Bacc("TRN2", debug=False)
    data_d = nc.dram_tensor("data", (BPC, N, D), F32, kind="ExternalInput")
    targ_d = nc.dram_tensor("targets", (BPC, N), F32, kind="ExternalInput")
    pool_d = nc.dram_tensor("task_pool", (T, D), F32, kind="ExternalInput")
    out_d = nc.dram_tensor("out", (BPC, N), F32, kind="ExternalOutput")

    with tile.TileContext(nc) as tc:
        with tc.tile_pool(name="const", bufs=1) as const:
            utri = const.tile([PB, PB], F32R)     # strictly-upper ones (lhsT)
            onesm = const.tile([PB, PB], F32R)    # all-ones
            waug = const.tile([D + 1, T], F32R)        # [W ; -1]
            daug = const.tile([D + 1, BPC * N], F32R)  # [data.T ; tgt]
            den = [const.tile([PB, BPC], F32, name=f"den{c}", tag=f"den{c}") for c in range(NCH)]
            num = [const.tile([PB, BPC], F32, name=f"num{c}", tag=f"num{c}") for c in range(NCH)]

            nc.any.memset(onesm[:].bitcast(F32), 1.0)
            nc.any.memset(waug[D : D + 1, :].bitcast(F32), -1.0)

            # ---- setup: transpose task_pool and data into lhsT layouts ----
            with (
                tc.tile_pool(name="ld", bufs=1) as ld,
                tc.tile_pool(name="tps", bufs=4, space="PSUM") as tps,
            ):
                ident = ld.tile([PB, PB], F32, tag="ident", name="ident")
                masks.make_identity(nc, ident[:])
                utri_f = ld.tile([PB, PB], F32, tag="utri_f", name="utri_f")
                masks.make_upper_triangular(nc, utri_f[:], 1.0, diag=False)
                nc.vector.tensor_copy(utri[:], utri_f[:])
                tstag = ld.tile([1, BPC * N], F32, tag="tstag", name="tstag")
                wbig = ld.tile([PB, (T // PB) * D], F32, tag="wbig", name="wbig")
                nc.sync.dma_start(
                    wbig[:].rearrange("p (k d) -> p k d", d=D),
                    pool_d[:].rearrange("(k p) d -> p k d", p=PB),
                )
                TB = 4  # transposes batched per PSUM tile
                # one contiguous targets DMA; daug row D gates the first resid
                nc.sync.dma_start(
                    tstag[:],
                    targ_d[:].rearrange("b n -> (b n)").rearrange(
                        "(one m) -> one m", one=1
                    ),
                )
                nc.scalar.activation(
                    daug[D : D + 1, 0:N], tstag[:, 0:N], AF.Copy
                )
                dbigs = []
                for b in range(BPC):
                    dbig = ld.tile(
                        [PB, NCH * D], F32, tag=f"dbig{b}", name="dbig"
                    )
                    nc.sync.dma_start(
                        dbig[:].rearrange("p (c d) -> p c d", d=D),
                        data_d[b].rearrange("(c p) d -> p c d", p=PB),
                    )
                    dbigs.append(dbig)
                def dtrans(g, eng):
                    pt = tps.tile([D, TB * PB], F32, tag="pt", name="pt")
                    for i in range(TB):
                        bc = g * TB + i
                        b, c = bc // NCH, bc % NCH
                        nc.tensor.transpose(
                            pt[:, i * PB : (i + 1) * PB],
                            dbigs[b][:, c * D : (c + 1) * D],
                            ident[:],
                        )
                    dst = daug[0:D, g * TB * PB : (g + 1) * TB * PB]
                    if eng == "act":
                        nc.scalar.activation(dst, pt[:], AF.Copy)
                    else:
                        nc.vector.tensor_copy(dst, pt[:])

                def wtrans(g, eng):
                    pt = tps.tile([D, TB * PB], F32, tag="pt", name="pt")
                    for i in range(TB):
                        k = g * TB + i
                        nc.tensor.transpose(
                            pt[:, i * PB : (i + 1) * PB],
                            wbig[:, k * D : (k + 1) * D],
                            ident[:],
                        )
                    dst = waug[0:D, g * TB * PB : (g + 1) * TB * PB]
                    if eng == "act":
                        nc.scalar.activation(dst, pt[:], AF.Copy)
                    else:
                        nc.vector.tensor_copy(dst, pt[:])

                # earliest-needed first, copies alternating Act/DVE:
                # b0 needs daug g0, waug g0-1; later groups feed later rounds
                dtrans(0, "act")
                wtrans(0, "dve")
                nc.scalar.activation(
                    daug[D : D + 1, N:], tstag[:, N:], AF.Copy
                )
                wtrans(1, "act")
                dtrans(1, "dve")
                wtrans(2, "act")
                wtrans(3, "dve")
                dtrans(2, "act")
                wtrans(4, "dve")
                wtrans(5, "act")
                dtrans(3, "dve")
                wtrans(6, "act")
                wtrans(7, "dve")

            # ---- main pipeline ----
            with (
                tc.tile_pool(name="sqp", bufs=6) as sqp,
                tc.tile_pool(name="avp", bufs=3) as avp,
                tc.tile_pool(name="ep", bufs=4) as ep,
                tc.tile_pool(name="small", bufs=4) as small,
                tc.tile_pool(name="rpp", bufs=2, space="PSUM") as rpp,
                tc.tile_pool(name="spp", bufs=2, space="PSUM") as spp,
            ):

                def s1_alloc(b):
                    av = [
                        avp.tile([PB, T], F32, tag=f"av{c}", name=f"av{c}")
                        for c in range(NCH)
                    ]
                    mx4 = [
                        small.tile([PB, NJT], F32, tag=f"mx4{c}", name=f"mx4{c}")
                        for c in range(NCH)
                    ]
                    return av, mx4

                def s1_resid(b, jt, c):
                    """resid matmul -> PSUM; Act square (fused PSUM egress)."""
                    cs = slice(b * N + c * PB, b * N + (c + 1) * PB)
                    rp = rpp.tile([PB, PT], F32, tag="rp", name="rp")
                    for h in range(NMM):
                        lo_ = jt * PT + h * MT
                        nc.tensor.matmul(
                            rp[:, h * MT : (h + 1) * MT],
                            daug[:, cs],
                            waug[:, lo_ : lo_ + MT],
                            start=True,
                            stop=True,
                        )
                    sq = sqp.tile([PB, PT], F32R, tag=f"sq{c}", name=f"sq{c}")
                    nc.scalar.activation(sq[:], rp[:], AF.Square)
                    return sq

                def s1_cum(b, jt, c, sqs, av, mx4):
                    """strict cumsum + evac(-0.5, fused row-max).

                    One evac slot per b goes Act-Copy(-0.5) + DVE in-place
                    SBUF max (2x mode, 594ns) to balance Act/DVE load."""
                    js = slice(jt * PT, (jt + 1) * PT)
                    sp = spp.tile([PB, PT], F32, tag="sp", name="sp")
                    for h in range(NMM):
                        hsl = slice(h * MT, (h + 1) * MT)
                        nc.tensor.matmul(
                            sp[:, hsl], utri[:], sqs[c][:, hsl],
                            start=True, stop=(c == 0),
                        )
                        if c == 1:
                            nc.tensor.matmul(
                                sp[:, hsl], onesm[:], sqs[0][:, hsl],
                                start=False, stop=True,
                            )
                    if jt == 1 and c == 0:
                        nc.scalar.activation(
                            av[c][:, js], sp[:], AF.Copy, scale=-0.5
                        )
                        nc.vector.tensor_scalar(
                            out=av[c][:, js],
                            in0=av[c][:, js],
                            scalar1=1.0,
                            scalar2=None,
                            op0=OP.mult,
                            op1=OP.max,
                            accum_out=mx4[c][:, jt : jt + 1],
                        )
                    else:
                        nc.vector.tensor_scalar(
                            out=av[c][:, js],
                            in0=sp[:],
                            scalar1=-0.5,
                            scalar2=None,
                            op0=OP.mult,
                            op1=OP.max,
                            accum_out=mx4[c][:, jt : jt + 1],
                        )

                def s1_finish_c(b, mx4, c):
                    """negated row-max once chunk c's evac partials landed."""
                    nb = small.tile([PB, 1], F32, tag=f"nb{c}", name=f"nb{c}")
                    nc.vector.tensor_reduce(
                        nb[:], mx4[c][:], axis=mybir.AxisListType.X, op=OP.max,
                        negate=True,
                    )
                    return nb

                def s2_exp(b, jt, c, av, nbs, den4):
                    """E = exp(av - rowmax) -> bf16, den partial via accum."""
                    js = slice(jt * PT, (jt + 1) * PT)
                    ev = ep.tile([PB, PT], BF16, tag=f"E{c}", name=f"E{c}")
                    nc.scalar.activation(
                        ev[:],
                        av[c][:, js],
                        AF.Exp,
                        bias=nbs[c][:],
                        scale=1.0,
                        accum_out=den4[c][:, jt : jt + 1],
                    )
                    return ev

                def s2_num(b, jt, c, ev, num4):
                    """Z recompute (K=64) + fused E*Z row-sum on DVE."""
                    rp2 = spp.tile([PB, PT], F32, tag="sp", name="rp2")
                    lhsT_z = daug[0:D, b * N + c * PB : b * N + (c + 1) * PB]
                    for h in range(NMM):
                        lo_ = jt * PT + h * MT
                        nc.tensor.matmul(
                            rp2[:, h * MT : (h + 1) * MT],
                            lhsT_z,
                            waug[0:D, lo_ : lo_ + MT],
                        )
                    nc.vector.scalar_tensor_tensor(
                        out=ev[:],
                        in0=rp2[:],
                        scalar=1.0,
                        in1=ev[:],
                        op0=OP.mult,
                        op1=OP.mult,
                        accum_out=num4[c][:, jt : jt + 1],
                    )

                def s2_alloc(b):
                    den4 = [
                        small.tile([PB, NJT], F32, tag=f"den4{c}", name=f"den4{c}")
                        for c in range(NCH)
                    ]
                    num4 = [
                        small.tile([PB, NJT], F32, tag=f"num4{c}", name=f"num4{c}")
                        for c in range(NCH)
                    ]
                    return den4, num4

                def s2_finish(b, den4, num4):
                    for c in range(NCH):
                        nc.vector.tensor_reduce(
                            den[c][:, b : b + 1], den4[c][:],
                            axis=mybir.AxisListType.X, op=OP.add,
                        )
                        nc.vector.tensor_reduce(
                            num[c][:, b : b + 1], num4[c][:],
                            axis=mybir.AxisListType.X, op=OP.add,
                        )

                # modulo-scheduled pipeline: per-jt rounds interleave batch b's
                # stage-1 chain with batch b-1's stage-2 chain so each engine's
                # in-order stream always has ready work at the front.
                def round_(b, jt, av, mx4, prev):
                    if prev is not None:
                        pb, pav, pnbs, pden4, pnum4 = prev
                    if jt == 0:
                        # new-b squares first: the old-b exp waits on the
                        # row-max reduce and must not block them on Act
                        sq0 = s1_resid(b, jt, 0)
                        sq1 = s1_resid(b, jt, 1)
                        if prev is not None:
                            ev0 = s2_exp(pb, jt, 0, pav, pnbs, pden4)
                            ev1 = s2_exp(pb, jt, 1, pav, pnbs, pden4)
                    else:
                        if prev is not None:
                            ev0 = s2_exp(pb, jt, 0, pav, pnbs, pden4)
                        sq0 = s1_resid(b, jt, 0)
                        if prev is not None:
                            ev1 = s2_exp(pb, jt, 1, pav, pnbs, pden4)
                        sq1 = s1_resid(b, jt, 1)
                    sqs = [sq0, sq1]
                    if prev is not None:
                        s2_num(pb, jt, 0, ev0, pnum4)
                    s1_cum(b, jt, 0, sqs, av, mx4)
                    nbs_out = []
                    if jt == NJT - 1:
                        nbs_out.append(s1_finish_c(b, mx4, 0))
                    if prev is not None:
                        s2_num(pb, jt, 1, ev1, pnum4)
                    s1_cum(b, jt, 1, sqs, av, mx4)
                    if jt == NJT - 1:
                        nbs_out.append(s1_finish_c(b, mx4, 1))
                    return nbs_out

                prev = None
                for b in range(BPC):
                    av, mx4 = s1_alloc(b)
                    for jt in range(NJT):
                        nbs_last = round_(b, jt, av, mx4, prev)
                    if prev is not None:
                        s2_finish(prev[0], prev[3], prev[4])
                    nbs = nbs_last
                    den4, num4 = s2_alloc(b)
                    prev = (b, av, nbs, den4, num4)
                pb, pav, pnbs, pden4, pnum4 = prev
                for jt in range(NJT):
                    ev0 = s2_exp(pb, jt, 0, pav, pnbs, pden4)
                    s2_num(pb, jt, 0, ev0, pnum4)
                    ev1 = s2_exp(pb, jt, 1, pav, pnbs, pden4)
                    s2_num(pb, jt, 1, ev1, pnum4)
                s2_finish(pb, pden4, pnum4)

                # finals: out = num/den
                for c in range(NCH):
                    rec = small.tile([PB, BPC], F32, tag=f"rec{c}", name=f"rec{c}")
                    outv = small.tile([PB, BPC], F32, tag=f"outv{c}", name=f"outv{c}")
                    nc.vector.reciprocal(rec[:], den[c][:])
                    nc.vector.tensor_mul(outv[:], num[c][:], rec[:])
                    ov = out_d[:, c * PB : (c + 1) * PB].rearrange("b p -> p b")
                    nc.sync.dma_start(ov, outv[:])

    nc.compile()
    return nc


def _get_nc():
    global _cached_nc
    if _cached_nc is None:
        _cached_nc = _build()
    return _cached_nc


_cached_runner = None


def _get_runner():
    """Build once: a cached jax.jit shard_map over the 8 NeuronCores.

    run_bass_kernel_spmd/run_bass_via_pjrt construct a fresh jax.jit closure
    per call (full retrace); caching the callable keeps repeat calls cheap.
    """
    global _cached_runner
    if _cached_runner is None:
        import jax
        from jax.sharding import Mesh, PartitionSpec
        from concourse import bass2jax
        from concourse.bass2jax import _bass_exec_p, partition_id_tensor
        import concourse.mybir as mybir

        try:
            from jax.experimental.shard_map import shard_map
        except ImportError:
            from jax.shard_map import shard_map

        bass2jax.install_neuronx_cc_hook()
        nc = _get_nc()
        partition_name = (
            nc.partition_id_tensor.name if nc.partition_id_tensor else None
        )
        in_names, out_names, out_avals, zero_outs = [], [], [], []
        for alloc in nc.m.functions[0].allocations:
            if not isinstance(alloc, mybir.MemoryLocationSet):
                continue
            name = alloc.memorylocations[0].name
            if alloc.kind == "ExternalInput":
                if name != partition_name:
                    in_names.append(name)
            elif alloc.kind == "ExternalOutput":
                out_names.append(name)
                shape = tuple(alloc.tensor_shape)
                dtype = mybir.dt.np(alloc.dtype)
                out_avals.append(jax.core.ShapedArray(shape, dtype))
                zero_outs.append(np.zeros((NCORES * shape[0], *shape[1:]), dtype))
        n_params = len(in_names)
        all_names = list(in_names) + list(out_names)
        if partition_name is not None:
            all_names.append(partition_name)
        donate = tuple(range(n_params, n_params + len(out_names)))

        def _body(*args):
            operands = list(args)
            if partition_name is not None:
                operands.append(partition_id_tensor())
            return tuple(
                _bass_exec_p.bind(
                    *operands,
                    out_avals=tuple(out_avals),
                    in_names=tuple(all_names),
                    out_names=tuple(out_names),
                    lowering_input_output_aliases=(),
                    sim_require_finite=True,
                    sim_require_nnan=True,
                    nc=nc,
                )
            )

        devices = jax.devices()[:NCORES]
        mesh = Mesh(np.asarray(devices), ("core",))
        in_specs = tuple(
            PartitionSpec() if name == "task_pool" else PartitionSpec("core")
            for name in in_names
        ) + (PartitionSpec("core"),) * len(out_names)
        sharded = jax.jit(
            shard_map(
                _body,
                mesh=mesh,
                in_specs=in_specs,
                out_specs=(PartitionSpec("core"),) * len(out_names),
                check_rep=False,
            ),
            donate_argnums=donate,
            keep_unused=True,
        )
        _cached_runner = (sharded, in_names, out_names, out_avals, zero_outs)
    return _cached_runner


def _kernel_fallback(data, targets, tp):
    """Robust path via the stock SPMD runner (fresh jit each call)."""
    from concourse.bass_utils import run_bass_kernel_spmd

    nc = _get_nc()
    in_maps = [
        {
            "data": data[i * BPC : (i + 1) * BPC],
            "targets": targets[i * BPC : (i + 1) * BPC],
            "task_pool": tp,
        }
        for i in range(NCORES)
    ]
    res = run_bass_kernel_spmd(nc, in_maps, core_ids=list(range(NCORES)))
    return np.concatenate([r["out"] for r in res.results], axis=0)


def kernel(data, targets, task_pool, **_):
    data = np.ascontiguousarray(np.asarray(data, np.float32))
    targets = np.ascontiguousarray(np.asarray(targets, np.float32))
    tp = np.ascontiguousarray(np.asarray(task_pool, np.float32).reshape(T, D))

    try:
        sharded, in_names, out_names, out_avals, zero_outs = _get_runner()
        full = {
            "data": data.reshape(NCORES * BPC, N, D),
            "targets": targets.reshape(NCORES * BPC, N),
            "task_pool": tp,
        }
        args = [full[name] for name in in_names]
        args += [np.zeros_like(z) for z in zero_outs]
        outs = sharded(*args)
        out = np.asarray(outs[out_names.index("out")])
        return out.reshape(B, N)
    except Exception:
        return _kernel_fallback(data, targets, tp)


# revision 20
# speedup vs baseline: 1.5790x; 1.0137x over previous
"""DiscreteMMSE Trainium2 Bass kernel.

Math (per batch row b):
  Z = data[b] @ W                      [N, T]   (W = squeeze(task_pool).T)
  resid = Z - targets[b][:, None]      [N, T]
  S'[i] = sum_{n<i} resid[n]^2         (strict cumsum over N; S'[0] = 0)
  E = exp(-0.5*S' - max_t(-0.5*S'))    (softmax-stable weights)
  out[b, i] = (sum_t E[i]*Z[i]) / (sum_t E[i])

Identical to the reference softmax-posterior MMSE prediction: the Gaussian
log-pdf constant and common shifts cancel in the softmax; pred is the
posterior-weighted mean of the per-task predictions Z. Row 0 (uniform prior
over tasks) falls out of the strict cumsum (S'[0] = 0 => uniform weights).

Numerics: plain f32r (TF32-like) matmuls throughout. Measured end-to-end
rel_l2 = 5.7e-3 on device vs the fp32 reference (tolerance 2e-2): the f32r
input rounding perturbs logits by ~+-0.4 which the 4096-task posterior
average absorbs. This halves TensorE work and removes all hi/lo split traffic
(~200us of Pool/DVE busy) vs the exact-fp32 variant.

Hardware constraints (verified against the BIR verifier) that dictate the
engine split: GPSIMD/Pool cannot touch PSUM at all; DVE cannot read two
PSUM operands (so it cannot square a PSUM tile); DMA cannot address PSUM;
only Act can square straight out of PSUM; f32r matmul inputs must come
from rounding-capable producers (engine cast copies - never DMA/bitcast).
Six [128,1024] PSUM tiles (resid x2, cumsum x2, Z x2) must therefore be
egressed per round by Act+DVE alone - each egress is fused with its
compute so no pass is pure data movement.

Layout per NeuronCore (pure data parallel over B: 8 rows each, no
collectives): N=256 steps on partitions as two 128-row chunks (c=0,1);
T=4096 tasks on the free dim in four 1024-col tiles (jt). Per round
(both chunks of one jt), cost-model busy ns:
  PE   : resid matmuls (lhsT=[data.T;tgt], rhs=[W;-1], K=65, f32r),
         strict-cumsum via triangular matmul (+ ones-matmul chunk0
         column-sum offset into chunk1), Z recompute (K=64)    [2989]
  Act  : Square c0+c1 (PSUM resid -> SBUF sq f32r); one batched
         [128,4096] Exp per (chunk, b) (av -> E bf16, bias=-rowmax,
         accum_out writes den[c][:,b] directly)                [~4343]
  DVE  : both evacs (PSUM cum * -0.5, fused row-max accum -> av),
         both scalar_tensor_tensor (E*Z fused with row-sum accum ->
         num partials; one instruction replaces mul+sum)       [~4800]
Modulo-scheduled: per-jt rounds interleave batch b's stage-1 chain with
batch b-1's stage-2 chain so each engine's in-order queue stays fed.
"""

import numpy as np

B, N, D, T = 64, 256, 64, 4096
NCORES = 8
BPC = B // NCORES  # batch rows per core
NCH = 2            # partition chunks of N
PB = 128           # partitions per chunk
PT = 1024          # psum tile free size (2 banks)
MT = 512           # matmul moving free size (1 bank)
NJT = T // PT      # psum tiles per chunk row
NMM = PT // MT     # matmuls per psum tile

_cached_nc = None


def _build():
    import concourse.bacc as bacc
    import concourse.mybir as mybir
    import concourse.tile as tile
    from concourse import masks

    F32 = mybir.dt.float32
    F32R = mybir.dt.float32r
    BF16 = mybir.dt.bfloat16
    AF = mybir.ActivationFunctionType
    OP = mybir.AluOpType

    nc = bacc.Bacc("TRN2", debug=False)
    data_d = nc.dram_tensor("data", (BPC, N, D), F32, kind="ExternalInput")
    targ_d = nc.dram_tensor("targets", (BPC, N), F32, kind="ExternalInput")
    pool_d = nc.dram_tensor("task_pool", (T, D), F32, kind="ExternalInput")
    out_d = nc.dram_tensor("out", (BPC, N), F32, kind="ExternalOutput")

    with tile.TileContext(nc) as tc:
        with tc.tile_pool(name="const", bufs=1) as const:
            utri = const.tile([PB, PB], F32R)     # strictly-upper ones (lhsT)
            onesm = const.tile([PB, PB], F32R)    # all-ones
            waug = const.tile([D + 1, T], F32R)        # [W ; -1]
            daug = const.tile([D + 1, BPC * N], F32R)  # [data.T ; tgt]
            den = [const.tile([PB, BPC], F32, name=f"den{c}", tag=f"den{c}") for c in range(NCH)]
            num = [const.tile([PB, BPC], F32, name=f"num{c}", tag=f"num{c}") for c in range(NCH)]

            nc.any.memset(onesm[:].bitcast(F32), 1.0)
            nc.any.memset(waug[D : D + 1, :].bitcast(F32), -1.0)

            # ---- setup: transpose task_pool and data into lhsT layouts ----
            with (
                tc.tile_pool(name="ld", bufs=1) as ld,
                tc.tile_pool(name="tps", bufs=4, space="PSUM") as tps,
            ):
                ident = ld.tile([PB, PB], F32, tag="ident", name="ident")
                masks.make_identity(nc, ident[:])
                utri_f = ld.tile([PB, PB], F32, tag="utri_f", name="utri_f")
                masks.make_upper_triangular(nc, utri_f[:], 1.0, diag=False)
                nc.vector.tensor_copy(utri[:], utri_f[:])
                tstag = ld.tile([1, BPC * N], F32, tag="tstag", name="tstag")
                wbig = ld.tile([PB, (T // PB) * D], F32, tag="wbig", name="wbig")
                nc.sync.dma_start(
                    wbig[:].rearrange("p (k d) -> p k d", d=D),
                    pool_d[:].rearrange("(k p) d -> p k d", p=PB),
                )
                TB = 4  # transposes batched per PSUM tile
                # one contiguous targets DMA; daug row D gates the first resid
                nc.sync.dma_start(
                    tstag[:],
                    targ_d[:].rearrange("b n -> (b n)").rearrange(
                        "(one m) -> one m", one=1
                    ),
                )
                nc.scalar.activation(
                    daug[D : D + 1, 0:N], tstag[:, 0:N], AF.Copy
                )
                dbigs = []
                for b in range(BPC):
                    dbig = ld.tile(
                        [PB, NCH * D], F32, tag=f"dbig{b}", name="dbig"
                    )
                    nc.sync.dma_start(
                        dbig[:].rearrange("p (c d) -> p c d", d=D),
                        data_d[b].rearrange("(c p) d -> p c d", p=PB),
                    )
                    dbigs.append(dbig)
                def dtrans(g, eng):
                    pt = tps.tile([D, TB * PB], F32, tag="pt", name="pt")
                    for i in range(TB):
                        bc = g * TB + i
                        b, c = bc // NCH, bc % NCH
                        nc.tensor.transpose(
                            pt[:, i * PB : (i + 1) * PB],
                            dbigs[b][:, c * D : (c + 1) * D],
                            ident[:],
                        )
                    dst = daug[0:D, g * TB * PB : (g + 1) * TB * PB]
                    if eng == "act":
                        nc.scalar.activation(dst, pt[:], AF.Copy)
                    else:
                        nc.vector.tensor_copy(dst, pt[:])

                def wtrans(g, eng):
                    pt = tps.tile([D, TB * PB], F32, tag="pt", name="pt")
                    for i in range(TB):
                        k = g * TB + i
                        nc.tensor.transpose(
                            pt[:, i * PB : (i + 1) * PB],
                            wbig[:, k * D : (k + 1) * D],
                            ident[:],
                        )
                    dst = waug[0:D, g * TB * PB : (g + 1) * TB * PB]
                    if eng == "act":
                        nc.scalar.activation(dst, pt[:], AF.Copy)
                    else:
                        nc.vector.tensor_copy(dst, pt[:])

                # earliest-needed first, copies alternating Act/DVE:
                # b0 needs daug g0, waug g0-1; later groups feed later rounds
                dtrans(0, "act")
                wtrans(0, "dve")
                nc.scalar.activation(
                    daug[D : D + 1, N:], tstag[:, N:], AF.Copy
                )
                wtrans(1, "act")
                dtrans(1, "dve")
                wtrans(2, "act")
                wtrans(3, "dve")
                dtrans(2, "act")
                wtrans(4, "dve")
                wtrans(5, "act")
                dtrans(3, "dve")
                wtrans(6, "act")
                wtrans(7, "dve")

            # ---- main pipeline ----
            with (
                tc.tile_pool(name="sqp", bufs=6) as sqp,
                tc.tile_pool(name="avp", bufs=3) as avp,
                tc.tile_pool(name="ep", bufs=4) as ep,
                tc.tile_pool(name="small", bufs=4) as small,
                tc.tile_pool(name="rpp", bufs=2, space="PSUM") as rpp,
                tc.tile_pool(name="spp", bufs=2, space="PSUM") as spp,
            ):

                def s1_alloc(b):
                    av = [
                        avp.tile([PB, T], F32, tag=f"av{c}", name=f"av{c}")
                        for c in range(NCH)
                    ]
                    mx4 = [
                        small.tile([PB, NJT], F32, tag=f"mx4{c}", name=f"mx4{c}")
                        for c in range(NCH)
                    ]
                    return av, mx4

                def s1_resid(b, jt, c):
                    """resid matmul -> PSUM; Act square (fused PSUM egress)."""
                    cs = slice(b * N + c * PB, b * N + (c + 1) * PB)
                    rp = rpp.tile([PB, PT], F32, tag="rp", name="rp")
                    for h in range(NMM):
                        lo_ = jt * PT + h * MT
                        nc.tensor.matmul(
                            rp[:, h * MT : (h + 1) * MT],
                            daug[:, cs],
                            waug[:, lo_ : lo_ + MT],
                            start=True,
                            stop=True,
                        )
                    sq = sqp.tile([PB, PT], F32R, tag=f"sq{c}", name=f"sq{c}")
                    nc.scalar.activation(sq[:], rp[:], AF.Square)
                    return sq

                def s1_cum(b, jt, c, sqs, av, mx4):
                    """strict cumsum + evac(-0.5, fused row-max).

                    One evac slot per b goes Act-Copy(-0.5) + DVE in-place
                    SBUF max (2x mode, 594ns) to balance Act/DVE load."""
                    js = slice(jt * PT, (jt + 1) * PT)
                    sp = spp.tile([PB, PT], F32, tag="sp", name="sp")
                    for h in range(NMM):
                        hsl = slice(h * MT, (h + 1) * MT)
                        nc.tensor.matmul(
                            sp[:, hsl], utri[:], sqs[c][:, hsl],
                            start=True, stop=(c == 0),
                        )
                        if c == 1:
                            nc.tensor.matmul(
                                sp[:, hsl], onesm[:], sqs[0][:, hsl],
                                start=False, stop=True,
                            )
                    if False:
                        pass
                    else:
                        nc.vector.tensor_scalar(
                            out=av[c][:, js],
                            in0=sp[:],
                            scalar1=-0.5,
                            scalar2=None,
                            op0=OP.mult,
                            op1=OP.max,
                            accum_out=mx4[c][:, jt : jt + 1],
                        )

                def s1_finish_c(b, mx4, c):
                    """negated row-max once chunk c's evac partials landed."""
                    nb = small.tile([PB, 1], F32, tag=f"nb{c}", name=f"nb{c}")
                    nc.vector.tensor_reduce(
                        nb[:], mx4[c][:], axis=mybir.AxisListType.X, op=OP.max,
                        negate=True,
                    )
                    return nb

                def s2_exp(b, jt, c, av, nbs, den4):
                    """E = exp(av - rowmax) -> bf16, den partial via accum."""
                    js = slice(jt * PT, (jt + 1) * PT)
                    ev = ep.tile([PB, PT], BF16, tag=f"E{c}", name=f"E{c}")
                    nc.scalar.activation(
                        ev[:],
                        av[c][:, js],
                        AF.Exp,
                        bias=nbs[c][:],
                        scale=1.0,
                        accum_out=den4[c][:, jt : jt + 1],
                    )
                    return ev

                def s2_num(b, jt, c, ev, num4):
                    """Z recompute (K=64) + fused E*Z row-sum on DVE."""
                    rp2 = spp.tile([PB, PT], F32, tag="sp", name="rp2")
                    lhsT_z = daug[0:D, b * N + c * PB : b * N + (c + 1) * PB]
                    for h in range(NMM):
                        lo_ = jt * PT + h * MT
                        nc.tensor.matmul(
                            rp2[:, h * MT : (h + 1) * MT],
                            lhsT_z,
                            waug[0:D, lo_ : lo_ + MT],
                        )
                    nc.vector.scalar_tensor_tensor(
                        out=ev[:],
                        in0=rp2[:],
                        scalar=1.0,
                        in1=ev[:],
                        op0=OP.mult,
                        op1=OP.mult,
                        accum_out=num4[c][:, jt : jt + 1],
                    )

                def s2_alloc(b):
                    den4 = [
                        small.tile([PB, NJT], F32, tag=f"den4{c}", name=f"den4{c}")
                        for c in range(NCH)
                    ]
                    num4 = [
                        small.tile([PB, NJT], F32, tag=f"num4{c}", name=f"num4{c}")
                        for c in range(NCH)
                    ]
                    return den4, num4

                def s2_finish(b, den4, num4):
                    for c in range(NCH):
                        nc.vector.tensor_reduce(
                            den[c][:, b : b + 1], den4[c][:],
                            axis=mybir.AxisListType.X, op=OP.add,
                        )
                        nc.vector.tensor_reduce(
                            num[c][:, b : b + 1], num4[c][:],
                            axis=mybir.AxisListType.X, op=OP.add,
                        )

                # modulo-scheduled pipeline: per-jt rounds interleave batch b's
                # stage-1 chain with batch b-1's stage-2 chain so each engine's
                # in-order stream always has ready work at the front.
                def round_(b, jt, av, mx4, prev):
                    if prev is not None:
                        pb, pav, pnbs, pden4, pnum4 = prev
                    if jt == 0:
                        # new-b squares first: the old-b exp waits on the
                        # row-max reduce and must not block them on Act
                        sq0 = s1_resid(b, jt, 0)
                        sq1 = s1_resid(b, jt, 1)
                        if prev is not None:
                            ev0 = s2_exp(pb, jt, 0, pav, pnbs, pden4)
                            ev1 = s2_exp(pb, jt, 1, pav, pnbs, pden4)
                    else:
                        if prev is not None:
                            ev0 = s2_exp(pb, jt, 0, pav, pnbs, pden4)
                        sq0 = s1_resid(b, jt, 0)
                        if prev is not None:
                            ev1 = s2_exp(pb, jt, 1, pav, pnbs, pden4)
                        sq1 = s1_resid(b, jt, 1)
                    sqs = [sq0, sq1]
                    if prev is not None:
                        s2_num(pb, jt, 0, ev0, pnum4)
                    s1_cum(b, jt, 0, sqs, av, mx4)
                    nbs_out = []
                    if jt == NJT - 1:
                        nbs_out.append(s1_finish_c(b, mx4, 0))
                    if prev is not None:
                        s2_num(pb, jt, 1, ev1, pnum4)
                    s1_cum(b, jt, 1, sqs, av, mx4)
                    if jt == NJT - 1:
                        nbs_out.append(s1_finish_c(b, mx4, 1))
                    return nbs_out

                prev = None
                for b in range(BPC):
                    av, mx4 = s1_alloc(b)
                    for jt in range(NJT):
                        nbs_last = round_(b, jt, av, mx4, prev)
                    if prev is not None:
                        s2_finish(prev[0], prev[3], prev[4])
                    nbs = nbs_last
                    den4, num4 = s2_alloc(b)
                    prev = (b, av, nbs, den4, num4)
                pb, pav, pnbs, pden4, pnum4 = prev
                for jt in range(NJT):
                    ev0 = s2_exp(pb, jt, 0, pav, pnbs, pden4)
                    s2_num(pb, jt, 0, ev0, pnum4)
                    ev1 = s2_exp(pb, jt, 1, pav, pnbs, pden4)
                    s2_num(pb, jt, 1, ev1, pnum4)
                s2_finish(pb, pden4, pnum4)

                # finals: out = num/den
                for c in range(NCH):
                    rec = small.tile([PB, BPC], F32, tag=f"rec{c}", name=f"rec{c}")
                    outv = small.tile([PB, BPC], F32, tag=f"outv{c}", name=f"outv{c}")
                    nc.vector.reciprocal(rec[:], den[c][:])
                    nc.vector.tensor_mul(outv[:], num[c][:], rec[:])
                    ov = out_d[:, c * PB : (c + 1) * PB].rearrange("b p -> p b")
                    nc.sync.dma_start(ov, outv[:])

    nc.compile()
    return nc


def _get_nc():
    global _cached_nc
    if _cached_nc is None:
        _cached_nc = _build()
    return _cached_nc


_cached_runner = None


def _get_runner():
    """Build once: a cached jax.jit shard_map over the 8 NeuronCores.

    run_bass_kernel_spmd/run_bass_via_pjrt construct a fresh jax.jit closure
    per call (full retrace); caching the callable keeps repeat calls cheap.
    """
    global _cached_runner
    if _cached_runner is None:
        import jax
        from jax.sharding import Mesh, PartitionSpec
        from concourse import bass2jax
        from concourse.bass2jax import _bass_exec_p, partition_id_tensor
        import concourse.mybir as mybir

        try:
            from jax.experimental.shard_map import shard_map
        except ImportError:
            from jax.shard_map import shard_map

        bass2jax.install_neuronx_cc_hook()
        nc = _get_nc()
        partition_name = (
            nc.partition_id_tensor.name if nc.partition_id_tensor else None
        )
        in_names, out_names, out_avals, zero_outs = [], [], [], []
        for alloc in nc.m.functions[0].allocations:
            if not isinstance(alloc, mybir.MemoryLocationSet):
                continue
            name = alloc.memorylocations[0].name
            if alloc.kind == "ExternalInput":
                if name != partition_name:
                    in_names.append(name)
            elif alloc.kind == "ExternalOutput":
                out_names.append(name)
                shape = tuple(alloc.tensor_shape)
                dtype = mybir.dt.np(alloc.dtype)
                out_avals.append(jax.core.ShapedArray(shape, dtype))
                zero_outs.append(np.zeros((NCORES * shape[0], *shape[1:]), dtype))
        n_params = len(in_names)
        all_names = list(in_names) + list(out_names)
        if partition_name is not None:
            all_names.append(partition_name)
        donate = tuple(range(n_params, n_params + len(out_names)))

        def _body(*args):
            operands = list(args)
            if partition_name is not None:
                operands.append(partition_id_tensor())
            return tuple(
                _bass_exec_p.bind(
                    *operands,
                    out_avals=tuple(out_avals),
                    in_names=tuple(all_names),
                    out_names=tuple(out_names),
                    lowering_input_output_aliases=(),
                    sim_require_finite=True,
                    sim_require_nnan=True,
                    nc=nc,
                )
            )

        devices = jax.devices()[:NCORES]
        mesh = Mesh(np.asarray(devices), ("core",))
        in_specs = tuple(
            PartitionSpec() if name == "task_pool" else PartitionSpec("core")
            for name in in_names
        ) + (PartitionSpec("core"),) * len(out_names)
        sharded = jax.jit(
            shard_map(
                _body,
                mesh=mesh,
                in_specs=in_specs,
                out_specs=(PartitionSpec("core"),) * len(out_names),
                check_rep=False,
            ),
            donate_argnums=donate,
            keep_unused=True,
        )
        _cached_runner = (sharded, in_names, out_names, out_avals, zero_outs)
    return _cached_runner


def _kernel_fallback(data, targets, tp):
    """Robust path via the stock SPMD runner (fresh jit each call)."""
    from concourse.bass_utils import run_bass_kernel_spmd

    nc = _get_nc()
    in_maps = [
        {
            "data": data[i * BPC : (i + 1) * BPC],
            "targets": targets[i * BPC : (i + 1) * BPC],
            "task_pool": tp,
        }
        for i in range(NCORES)
    ]
    res = run_bass_kernel_spmd(nc, in_maps, core_ids=list(range(NCORES)))
    return np.concatenate([r["out"] for r in res.results], axis=0)


def kernel(data, targets, task_pool, **_):
    data = np.ascontiguousarray(np.asarray(data, np.float32))
    targets = np.ascontiguousarray(np.asarray(targets, np.float32))
    tp = np.ascontiguousarray(np.asarray(task_pool, np.float32).reshape(T, D))

    try:
        sharded, in_names, out_names, out_avals, zero_outs = _get_runner()
        full = {
            "data": data.reshape(NCORES * BPC, N, D),
            "targets": targets.reshape(NCORES * BPC, N),
            "task_pool": tp,
        }
        args = [full[name] for name in in_names]
        args += [np.zeros_like(z) for z in zero_outs]
        outs = sharded(*args)
        out = np.asarray(outs[out_names.index("out")])
        return out.reshape(B, N)
    except Exception:
        return _kernel_fallback(data, targets, tp)


# revision 23
# speedup vs baseline: 1.5858x; 1.0043x over previous
"""DiscreteMMSE Trainium2 Bass kernel.

Math (per batch row b):
  Z = data[b] @ W                      [N, T]   (W = squeeze(task_pool).T)
  resid = Z - targets[b][:, None]      [N, T]
  S'[i] = sum_{n<i} resid[n]^2         (strict cumsum over N; S'[0] = 0)
  E = exp(-0.5*S' - max_t(-0.5*S'))    (softmax-stable weights)
  out[b, i] = (sum_t E[i]*Z[i]) / (sum_t E[i])

Identical to the reference softmax-posterior MMSE prediction: the Gaussian
log-pdf constant and common shifts cancel in the softmax; pred is the
posterior-weighted mean of the per-task predictions Z. Row 0 (uniform prior
over tasks) falls out of the strict cumsum (S'[0] = 0 => uniform weights).

Numerics: plain f32r (TF32-like) matmuls throughout. Measured end-to-end
rel_l2 = 5.7e-3 on device vs the fp32 reference (tolerance 2e-2): the f32r
input rounding perturbs logits by ~+-0.4 which the 4096-task posterior
average absorbs. This halves TensorE work and removes all hi/lo split traffic
(~200us of Pool/DVE busy) vs the exact-fp32 variant.

Hardware constraints (verified against the BIR verifier) that dictate the
engine split: GPSIMD/Pool cannot touch PSUM at all; DVE cannot read two
PSUM operands (so it cannot square a PSUM tile); DMA cannot address PSUM;
only Act can square straight out of PSUM; f32r matmul inputs must come
from rounding-capable producers (engine cast copies - never DMA/bitcast).
Six [128,1024] PSUM tiles (resid x2, cumsum x2, Z x2) must therefore be
egressed per round by Act+DVE alone - each egress is fused with its
compute so no pass is pure data movement.

Layout per NeuronCore (pure data parallel over B: 8 rows each, no
collectives): N=256 steps on partitions as two 128-row chunks (c=0,1);
T=4096 tasks on the free dim in four 1024-col tiles (jt). Per round
(both chunks of one jt), cost-model busy ns:
  PE   : resid matmuls (lhsT=[data.T;tgt], rhs=[W;-1], K=65, f32r),
         strict-cumsum via triangular matmul (+ ones-matmul chunk0
         column-sum offset into chunk1), Z recompute (K=64)    [2989]
  Act  : Square c0+c1 (PSUM resid -> SBUF sq f32r); one batched
         [128,4096] Exp per (chunk, b) (av -> E bf16, bias=-rowmax,
         accum_out writes den[c][:,b] directly)                [~4343]
  DVE  : both evacs (PSUM cum * -0.5, fused row-max accum -> av),
         both scalar_tensor_tensor (E*Z fused with row-sum accum ->
         num partials; one instruction replaces mul+sum)       [~4800]
Modulo-scheduled: per-jt rounds interleave batch b's stage-1 chain with
batch b-1's stage-2 chain so each engine's in-order queue stays fed.
"""

import numpy as np

B, N, D, T = 64, 256, 64, 4096
NCORES = 8
BPC = B // NCORES  # batch rows per core
NCH = 2            # partition chunks of N
PB = 128           # partitions per chunk
PT = 1024          # psum tile free size (2 banks)
MT = 512           # matmul moving free size (1 bank)
NJT = T // PT      # psum tiles per chunk row
NMM = PT // MT     # matmuls per psum tile

_cached_nc = None


def _build():
    import concourse.bacc as bacc
    import concourse.mybir as mybir
    import concourse.tile as tile
    from concourse import masks

    F32 = mybir.dt.float32
    F32R = mybir.dt.float32r
    BF16 = mybir.dt.bfloat16
    AF = mybir.ActivationFunctionType
    OP = mybir.AluOpType

    nc = bacc.Bacc("TRN2", debug=False)
    data_d = nc.dram_tensor("data", (BPC, N, D), F32, kind="ExternalInput")
    targ_d = nc.dram_tensor("targets", (BPC, N), F32, kind="ExternalInput")
    pool_d = nc.dram_tensor("task_pool", (T, D), F32, kind="ExternalInput")
    out_d = nc.dram_tensor("out", (BPC, N), F32, kind="ExternalOutput")

    with tile.TileContext(nc) as tc:
        with tc.tile_pool(name="const", bufs=1) as const:
            utri = const.tile([PB, PB], F32R)     # strictly-upper ones (lhsT)
            onesm = const.tile([PB, PB], F32R)    # all-ones
            waug = const.tile([D + 1, T], F32R)        # [W ; -1]
            daug = const.tile([D + 1, BPC * N], F32R)  # [data.T ; tgt]
            den = [const.tile([PB, BPC], F32, name=f"den{c}", tag=f"den{c}") for c in range(NCH)]
            num = [const.tile([PB, BPC], F32, name=f"num{c}", tag=f"num{c}") for c in range(NCH)]

            nc.any.memset(onesm[:].bitcast(F32), 1.0)
            nc.any.memset(waug[D : D + 1, :].bitcast(F32), -1.0)

            # ---- setup: transpose task_pool and data into lhsT layouts ----
            with (
                tc.tile_pool(name="ld", bufs=1) as ld,
                tc.tile_pool(name="tps", bufs=4, space="PSUM") as tps,
            ):
                ident = ld.tile([PB, PB], F32, tag="ident", name="ident")
                masks.make_identity(nc, ident[:])
                utri_f = ld.tile([PB, PB], F32, tag="utri_f", name="utri_f")
                masks.make_upper_triangular(nc, utri_f[:], 1.0, diag=False)
                nc.vector.tensor_copy(utri[:], utri_f[:])
                tstag = ld.tile([1, BPC * N], F32, tag="tstag", name="tstag")
                wbig = ld.tile([PB, (T // PB) * D], F32, tag="wbig", name="wbig")
                TB = 4  # transposes batched per PSUM tile
                # Each DMA costs ~1us serialized (HWDGE hold + dge delay +
                # transfer). Order by first use: dbig b0/b1 + targets gate the
                # first transposes/resid; the wbig remainder feeds rounds
                # jt1-3 (t~8+); dbig b2..7 only feed b1+ rounds (t~20+).
                dbigs = [
                    ld.tile([PB, NCH * D], F32, tag=f"dbig{b}", name="dbig")
                    for b in range(BPC)
                ]

                def dbig_dma(b):
                    nc.sync.dma_start(
                        dbigs[b][:].rearrange("p (c d) -> p c d", d=D),
                        data_d[b].rearrange("(c p) d -> p c d", p=PB),
                    )

                dbig_dma(0)
                dbig_dma(1)
                nc.sync.dma_start(
                    tstag[:],
                    targ_d[:].rearrange("b n -> (b n)").rearrange(
                        "(one m) -> one m", one=1
                    ),
                )
                WS = 2 * TB * D  # wbig cols feeding waug groups 0-1
                nc.sync.dma_start(
                    wbig[:, :WS].rearrange("p (k d) -> p k d", d=D),
                    pool_d[0 : 2 * TB * PB].rearrange("(k p) d -> p k d", p=PB),
                )
                nc.sync.dma_start(
                    wbig[:, WS:].rearrange("p (k d) -> p k d", d=D),
                    pool_d[2 * TB * PB :].rearrange("(k p) d -> p k d", p=PB),
                )
                for b in range(2, BPC):
                    dbig_dma(b)
                nc.scalar.activation(
                    daug[D : D + 1, 0:N], tstag[:, 0:N], AF.Copy
                )
                def dtrans(g, eng):
                    pt = tps.tile([D, TB * PB], F32, tag="pt", name="pt")
                    for i in range(TB):
                        bc = g * TB + i
                        b, c = bc // NCH, bc % NCH
                        nc.tensor.transpose(
                            pt[:, i * PB : (i + 1) * PB],
                            dbigs[b][:, c * D : (c + 1) * D],
                            ident[:],
                        )
                    dst = daug[0:D, g * TB * PB : (g + 1) * TB * PB]
                    if eng == "act":
                        nc.scalar.activation(dst, pt[:], AF.Copy)
                    else:
                        nc.vector.tensor_copy(dst, pt[:])

                def wtrans(g, eng):
                    pt = tps.tile([D, TB * PB], F32, tag="pt", name="pt")
                    for i in range(TB):
                        k = g * TB + i
                        nc.tensor.transpose(
                            pt[:, i * PB : (i + 1) * PB],
                            wbig[:, k * D : (k + 1) * D],
                            ident[:],
                        )
                    dst = waug[0:D, g * TB * PB : (g + 1) * TB * PB]
                    if eng == "act":
                        nc.scalar.activation(dst, pt[:], AF.Copy)
                    else:
                        nc.vector.tensor_copy(dst, pt[:])

                # earliest-needed first, copies alternating Act/DVE:
                # b0 needs daug g0, waug g0-1; later groups feed later rounds
                dtrans(0, "act")
                wtrans(0, "dve")
                nc.scalar.activation(
                    daug[D : D + 1, N:], tstag[:, N:], AF.Copy
                )
                wtrans(1, "act")
                dtrans(1, "dve")
                wtrans(2, "act")
                wtrans(3, "dve")
                dtrans(2, "act")
                wtrans(4, "dve")
                wtrans(5, "act")
                dtrans(3, "dve")
                wtrans(6, "act")
                wtrans(7, "dve")

            # ---- main pipeline ----
            with (
                tc.tile_pool(name="sqp", bufs=6) as sqp,
                tc.tile_pool(name="avp", bufs=3) as avp,
                tc.tile_pool(name="ep", bufs=4) as ep,
                tc.tile_pool(name="small", bufs=4) as small,
                tc.tile_pool(name="rpp", bufs=2, space="PSUM") as rpp,
                tc.tile_pool(name="spp", bufs=2, space="PSUM") as spp,
            ):

                def s1_alloc(b):
                    av = [
                        avp.tile([PB, T], F32, tag=f"av{c}", name=f"av{c}")
                        for c in range(NCH)
                    ]
                    mx4 = [
                        small.tile([PB, NJT], F32, tag=f"mx4{c}", name=f"mx4{c}")
                        for c in range(NCH)
                    ]
                    return av, mx4

                def s1_resid(b, jt, c):
                    """resid matmul -> PSUM; Act square (fused PSUM egress)."""
                    cs = slice(b * N + c * PB, b * N + (c + 1) * PB)
                    rp = rpp.tile([PB, PT], F32, tag="rp", name="rp")
                    for h in range(NMM):
                        lo_ = jt * PT + h * MT
                        nc.tensor.matmul(
                            rp[:, h * MT : (h + 1) * MT],
                            daug[:, cs],
                            waug[:, lo_ : lo_ + MT],
                            start=True,
                            stop=True,
                        )
                    sq = sqp.tile([PB, PT], F32R, tag=f"sq{c}", name=f"sq{c}")
                    nc.scalar.activation(sq[:], rp[:], AF.Square)
                    return sq

                def s1_cum(b, jt, c, sqs, av, mx4):
                    """strict cumsum + evac(-0.5, fused row-max).

                    One evac slot per b goes Act-Copy(-0.5) + DVE in-place
                    SBUF max (2x mode, 594ns) to balance Act/DVE load."""
                    js = slice(jt * PT, (jt + 1) * PT)
                    sp = spp.tile([PB, PT], F32, tag="sp", name="sp")
                    for h in range(NMM):
                        hsl = slice(h * MT, (h + 1) * MT)
                        nc.tensor.matmul(
                            sp[:, hsl], utri[:], sqs[c][:, hsl],
                            start=True, stop=(c == 0),
                        )
                        if c == 1:
                            nc.tensor.matmul(
                                sp[:, hsl], onesm[:], sqs[0][:, hsl],
                                start=False, stop=True,
                            )
                    if False:
                        pass
                    else:
                        nc.vector.tensor_scalar(
                            out=av[c][:, js],
                            in0=sp[:],
                            scalar1=-0.5,
                            scalar2=None,
                            op0=OP.mult,
                            op1=OP.max,
                            accum_out=mx4[c][:, jt : jt + 1],
                        )

                def s1_finish_c(b, mx4, c):
                    """negated row-max once chunk c's evac partials landed."""
                    nb = small.tile([PB, 1], F32, tag=f"nb{c}", name=f"nb{c}")
                    nc.vector.tensor_reduce(
                        nb[:], mx4[c][:], axis=mybir.AxisListType.X, op=OP.max,
                        negate=True,
                    )
                    return nb

                def s2_exp(b, jt, c, av, nbs, den4):
                    """E = exp(av - rowmax) -> bf16, den partial via accum."""
                    js = slice(jt * PT, (jt + 1) * PT)
                    ev = ep.tile([PB, PT], BF16, tag=f"E{c}", name=f"E{c}")
                    nc.scalar.activation(
                        ev[:],
                        av[c][:, js],
                        AF.Exp,
                        bias=nbs[c][:],
                        scale=1.0,
                        accum_out=den4[c][:, jt : jt + 1],
                    )
                    return ev

                def s2_num(b, jt, c, ev, num4):
                    """Z recompute (K=64) + fused E*Z row-sum on DVE."""
                    rp2 = spp.tile([PB, PT], F32, tag="sp", name="rp2")
                    lhsT_z = daug[0:D, b * N + c * PB : b * N + (c + 1) * PB]
                    for h in range(NMM):
                        lo_ = jt * PT + h * MT
                        nc.tensor.matmul(
                            rp2[:, h * MT : (h + 1) * MT],
                            lhsT_z,
                            waug[0:D, lo_ : lo_ + MT],
                        )
                    nc.vector.scalar_tensor_tensor(
                        out=ev[:],
                        in0=rp2[:],
                        scalar=1.0,
                        in1=ev[:],
                        op0=OP.mult,
                        op1=OP.mult,
                        accum_out=num4[c][:, jt : jt + 1],
                    )

                def s2_alloc(b):
                    den4 = [
                        small.tile([PB, NJT], F32, tag=f"den4{c}", name=f"den4{c}")
                        for c in range(NCH)
                    ]
                    num4 = [
                        small.tile([PB, NJT], F32, tag=f"num4{c}", name=f"num4{c}")
                        for c in range(NCH)
                    ]
                    return den4, num4

                def s2_finish(b, den4, num4):
                    for c in range(NCH):
                        nc.vector.tensor_reduce(
                            den[c][:, b : b + 1], den4[c][:],
                            axis=mybir.AxisListType.X, op=OP.add,
                        )
                        nc.vector.tensor_reduce(
                            num[c][:, b : b + 1], num4[c][:],
                            axis=mybir.AxisListType.X, op=OP.add,
                        )

                # modulo-scheduled pipeline: per-jt rounds interleave batch b's
                # stage-1 chain with batch b-1's stage-2 chain so each engine's
                # in-order stream always has ready work at the front.
                def round_(b, jt, av, mx4, prev):
                    if prev is not None:
                        pb, pav, pnbs, pden4, pnum4 = prev
                    if jt == 0:
                        # new-b squares first: the old-b exp waits on the
                        # row-max reduce and must not block them on Act
                        sq0 = s1_resid(b, jt, 0)
                        sq1 = s1_resid(b, jt, 1)
                        if prev is not None:
                            ev0 = s2_exp(pb, jt, 0, pav, pnbs, pden4)
                            ev1 = s2_exp(pb, jt, 1, pav, pnbs, pden4)
                    else:
                        if prev is not None:
                            ev0 = s2_exp(pb, jt, 0, pav, pnbs, pden4)
                        sq0 = s1_resid(b, jt, 0)
                        if prev is not None:
                            ev1 = s2_exp(pb, jt, 1, pav, pnbs, pden4)
                        sq1 = s1_resid(b, jt, 1)
                    sqs = [sq0, sq1]
                    if prev is not None:
                        s2_num(pb, jt, 0, ev0, pnum4)
                    s1_cum(b, jt, 0, sqs, av, mx4)
                    nbs_out = []
                    if jt == NJT - 1:
                        nbs_out.append(s1_finish_c(b, mx4, 0))
                    if prev is not None:
                        s2_num(pb, jt, 1, ev1, pnum4)
                    s1_cum(b, jt, 1, sqs, av, mx4)
                    if jt == NJT - 1:
                        nbs_out.append(s1_finish_c(b, mx4, 1))
                    return nbs_out

                prev = None
                for b in range(BPC):
                    av, mx4 = s1_alloc(b)
                    for jt in range(NJT):
                        nbs_last = round_(b, jt, av, mx4, prev)
                    if prev is not None:
                        s2_finish(prev[0], prev[3], prev[4])
                    nbs = nbs_last
                    den4, num4 = s2_alloc(b)
                    prev = (b, av, nbs, den4, num4)
                pb, pav, pnbs, pden4, pnum4 = prev
                for jt in range(NJT):
                    ev0 = s2_exp(pb, jt, 0, pav, pnbs, pden4)
                    s2_num(pb, jt, 0, ev0, pnum4)
                    ev1 = s2_exp(pb, jt, 1, pav, pnbs, pden4)
                    s2_num(pb, jt, 1, ev1, pnum4)
                s2_finish(pb, pden4, pnum4)

                # finals: out = num/den
                for c in range(NCH):
                    rec = small.tile([PB, BPC], F32, tag=f"rec{c}", name=f"rec{c}")
                    outv = small.tile([PB, BPC], F32, tag=f"outv{c}", name=f"outv{c}")
                    nc.vector.reciprocal(rec[:], den[c][:])
                    nc.vector.tensor_mul(outv[:], num[c][:], rec[:])
                    ov = out_d[:, c * PB : (c + 1) * PB].rearrange("b p -> p b")
                    nc.sync.dma_start(ov, outv[:])

    nc.compile()
    return nc


def _get_nc():
    global _cached_nc
    if _cached_nc is None:
        _cached_nc = _build()
    return _cached_nc


_cached_runner = None


def _get_runner():
    """Build once: a cached jax.jit shard_map over the 8 NeuronCores.

    run_bass_kernel_spmd/run_bass_via_pjrt construct a fresh jax.jit closure
    per call (full retrace); caching the callable keeps repeat calls cheap.
    """
    global _cached_runner
    if _cached_runner is None:
        import jax
        from jax.sharding import Mesh, PartitionSpec
        from concourse import bass2jax
        from concourse.bass2jax import _bass_exec_p, partition_id_tensor
        import concourse.mybir as mybir

        try:
            from jax.experimental.shard_map import shard_map
        except ImportError:
            from jax.shard_map import shard_map

        bass2jax.install_neuronx_cc_hook()
        nc = _get_nc()
        partition_name = (
            nc.partition_id_tensor.name if nc.partition_id_tensor else None
        )
        in_names, out_names, out_avals, zero_outs = [], [], [], []
        for alloc in nc.m.functions[0].allocations:
            if not isinstance(alloc, mybir.MemoryLocationSet):
                continue
            name = alloc.memorylocations[0].name
            if alloc.kind == "ExternalInput":
                if name != partition_name:
                    in_names.append(name)
            elif alloc.kind == "ExternalOutput":
                out_names.append(name)
                shape = tuple(alloc.tensor_shape)
                dtype = mybir.dt.np(alloc.dtype)
                out_avals.append(jax.core.ShapedArray(shape, dtype))
                zero_outs.append(np.zeros((NCORES * shape[0], *shape[1:]), dtype))
        n_params = len(in_names)
        all_names = list(in_names) + list(out_names)
        if partition_name is not None:
            all_names.append(partition_name)
        donate = tuple(range(n_params, n_params + len(out_names)))

        def _body(*args):
            operands = list(args)
            if partition_name is not None:
                operands.append(partition_id_tensor())
            return tuple(
                _bass_exec_p.bind(
                    *operands,
                    out_avals=tuple(out_avals),
                    in_names=tuple(all_names),
                    out_names=tuple(out_names),
                    lowering_input_output_aliases=(),
                    sim_require_finite=True,
                    sim_require_nnan=True,
                    nc=nc,
                )
            )

        devices = jax.devices()[:NCORES]
        mesh = Mesh(np.asarray(devices), ("core",))
        in_specs = tuple(
            PartitionSpec() if name == "task_pool" else PartitionSpec("core")
            for name in in_names
        ) + (PartitionSpec("core"),) * len(out_names)
        sharded = jax.jit(
            shard_map(
                _body,
                mesh=mesh,
                in_specs=in_specs,
                out_specs=(PartitionSpec("core"),) * len(out_names),
                check_rep=False,
            ),
            donate_argnums=donate,
            keep_unused=True,
        )
        _cached_runner = (sharded, in_names, out_names, out_avals, zero_outs)
    return _cached_runner


def _kernel_fallback(data, targets, tp):
    """Robust path via the stock SPMD runner (fresh jit each call)."""
    from concourse.bass_utils import run_bass_kernel_spmd

    nc = _get_nc()
    in_maps = [
        {
            "data": data[i * BPC : (i + 1) * BPC],
            "targets": targets[i * BPC : (i + 1) * BPC],
            "task_pool": tp,
        }
        for i in range(NCORES)
    ]
    res = run_bass_kernel_spmd(nc, in_maps, core_ids=list(range(NCORES)))
    return np.concatenate([r["out"] for r in res.results], axis=0)


def kernel(data, targets, task_pool, **_):
    data = np.ascontiguousarray(np.asarray(data, np.float32))
    targets = np.ascontiguousarray(np.asarray(targets, np.float32))
    tp = np.ascontiguousarray(np.asarray(task_pool, np.float32).reshape(T, D))

    try:
        sharded, in_names, out_names, out_avals, zero_outs = _get_runner()
        full = {
            "data": data.reshape(NCORES * BPC, N, D),
            "targets": targets.reshape(NCORES * BPC, N),
            "task_pool": tp,
        }
        args = [full[name] for name in in_names]
        args += [np.zeros_like(z) for z in zero_outs]
        outs = sharded(*args)
        out = np.asarray(outs[out_names.index("out")])
        return out.reshape(B, N)
    except Exception:
        return _kernel_fallback(data, targets, tp)
